# revision 18
# baseline (speedup 1.0000x reference)
"""AdaptiveSAGE GNN message-passing kernel for 8 Trainium2 NeuronCores.

Distribution strategy (dst-sharded message passing, PE-based segment sum):
  - Subgraph nodes padded to N_PAD = 81920 = 8 * 10240; core c owns rows
    [c*10240, (c+1)*10240).
  - The replicated H table is split into 4 quarter-tables of 20480 rows
    (so dma_gather's int16 indices always fit); a host-side node
    permutation maps core c's local rows [q*2560, (q+1)*2560) to quarter
    q at offset c*2560, which makes each quarter exactly one AllGather
    whose input is ready as soon as a quarter of the windows is computed
    -- three of the four AllGathers hide under compute.
  - Edges are assigned to the core owning their destination, bucketed by
    (src quarter, dst window of 128 rows), padded per bucket to a multiple
    of 128 so all cores run one identical instruction stream.
  - Messages are fetched with dma_gather in 1024-index chunks spread over
    the 4 SWDGE queues (all gpsimd Q7 pairs generate descriptors).
  - Segment-sum by destination runs on the TensorEngine: per 128-message
    tile a one-hot(dst) matrix (built by a batched DVE is_equal against an
    iota row) is the stationary matmul operand; one PSUM tile accumulates
    a whole window across all quarters (exact, no RMW races).
  - Messages / one-hots / H table are bf16 (halves gather and AllGather
    bytes, enables fast weight load); PSUM, weights, the local H, and the
    u,v head rows stay f32.
  - The tiny MLP heads (scores / halting probs on rows u=0, v=1) are
    evaluated on the host from the 5 x 2 x 128 head rows the kernel emits.
"""

import numpy as np

import concourse.bass as bass
import concourse.bacc as bacc
import concourse.tile as tile
import concourse.mybir as mybir
from concourse.bass_utils import run_bass_kernel_spmd

F = 128          # feature dim
N_CORES = 8
N_SUB = 80000
PER = 10240      # rows per core
N_PAD = N_CORES * PER
W = PER // F     # dst windows of 128 rows per core
NQ = 4           # quarter tables
QROWS = PER // NQ            # rows per core per quarter
QTAB = N_CORES * QROWS       # rows per quarter table (<= 32767 for int16)
QW = W // NQ                 # windows per quarter
LMAX = 5
CHUNK = 1024     # messages per gather chunk (SWDGE ring caps num_idxs ~<2K)
PAD_DL = 999.0   # out-of-window dst marker for padding slots


def _wrap16(idx: np.ndarray) -> np.ndarray:
    """SWDGE index layout: logical i -> [i%16, i//16], replicated across the
    8 groups of 16 partitions."""
    n = idx.shape[0]
    assert n % 16 == 0
    w = idx.reshape(n // 16, 16).T.astype(np.int16)
    return np.tile(w, (8, 1))


def _prep_edges(src: np.ndarray, dst: np.ndarray):
    """Bucket edges by (dst core, src quarter, dst window); pad each bucket
    to a common multiple-of-128 so the SPMD graph is uniform across cores.

    Gather indices address the permuted quarter tables: node id g with
    c = g // PER, r = g % PER lives in quarter r // QROWS at row
    c * QROWS + (r % QROWS).
    """
    assert QTAB <= 32768
    nw = W
    core_of = dst // PER
    src_c = src // PER
    src_r = src % PER
    bank_of = src_r // QROWS                    # quarter table
    src_idx = src_c * QROWS + (src_r % QROWS)   # row within quarter table
    dst_local = dst - core_of * PER
    w_of = dst_local // F
    run_of = bank_of * nw + w_of

    counts = np.zeros((N_CORES, NQ * nw), dtype=np.int64)
    per_core = []
    for c in range(N_CORES):
        m = core_of == c
        gl = src_idx[m].astype(np.int16)
        dl = (dst_local[m] % F).astype(np.float32)
        rid = run_of[m]
        order = np.argsort(rid, kind="stable")
        gl, dl, rid = gl[order], dl[order], rid[order]
        bounds = np.searchsorted(rid, np.arange(NQ * nw + 1))
        counts[c] = bounds[1:] - bounds[:-1]
        per_core.append((gl, dl, bounds))

    nt = np.ceil(counts.max(axis=0) / F).astype(np.int64)  # tiles per run
    # window-major run order (w, b): a window's bank runs are consecutive,
    # so one PSUM tile accumulates them all
    runs = [(w, b, int(nt[b * nw + w]))
            for w in range(nw) for b in range(NQ) if nt[b * nw + w] > 0]
    # per-bank padded slot counts (bank-major gather layout, window-sorted
    # within each bank) and per-run tile offsets in (w, b) order
    L = [0] * NQ
    slot0 = {}
    for b in range(NQ):
        for w in range(nw):
            n = int(nt[b * nw + w])
            if n:
                slot0[(b, w)] = L[b]
                L[b] += n * F
    tile0 = {}
    tg = 0
    for (w, b, n) in runs:
        tile0[(b, w)] = tg
        tg += n
    T_total = tg

    gidx, dls = [], []
    for c in range(N_CORES):
        gl, dl, bounds = per_core[c]
        gb = [np.zeros(L[b], np.int16) for b in range(NQ)]
        dla = np.full(T_total * F, PAD_DL, np.float32)
        for (w, b, n) in runs:
            r = b * nw + w
            seg = slice(bounds[r], bounds[r + 1])
            cnt = bounds[r + 1] - bounds[r]
            s0 = slot0[(b, w)]
            gb[b][s0:s0 + cnt] = gl[seg]
            t0 = tile0[(b, w)]
            dla[t0 * F:t0 * F + cnt] = dl[seg]
        gidx.append([_wrap16(x) for x in gb])
        dls.append(np.ascontiguousarray(dla.reshape(T_total, F).T))
    meta = dict(L=L, runs=runs, T_total=T_total, slot0=slot0, tile0=tile0)
    return gidx, dls, meta


def _build_graph(meta):
    """Build the SPMD Bass graph (identical for all 8 cores)."""
    L = meta["L"]
    runs = meta["runs"]
    T_total = meta["T_total"]
    slot0 = meta["slot0"]
    tile0 = meta["tile0"]
    f32 = mybir.dt.float32
    bf16 = mybir.dt.bfloat16
    i16 = mybir.dt.int16
    nc = bacc.Bacc("TRN2", target_bir_lowering=False, debug=False,
                   num_devices=N_CORES, num_swdge_queues=4)

    # ---- kernel I/O -------------------------------------------------------
    xT = nc.dram_tensor("xT", [F, PER], f32, kind="ExternalInput")
    invdeg = nc.dram_tensor("invdeg", [F, W], f32, kind="ExternalInput")
    winT = nc.dram_tensor("winT", [F, F], f32, kind="ExternalInput")
    wlT = nc.dram_tensor("wlT", [F, F], f32, kind="ExternalInput")
    wrT = nc.dram_tensor("wrT", [F, F], f32, kind="ExternalInput")
    bin_ = nc.dram_tensor("bin", [F, 1], f32, kind="ExternalInput")
    bl = nc.dram_tensor("bl", [F, 1], f32, kind="ExternalInput")
    ident = nc.dram_tensor("ident", [F, F], f32, kind="ExternalInput")
    iotar = nc.dram_tensor("iotar", [F, F], f32, kind="ExternalInput")
    dl_d = nc.dram_tensor("dl", [F, T_total], f32, kind="ExternalInput")
    gidx_d = [nc.dram_tensor(f"gidx{b}", [128, L[b] // 16], i16,
                             kind="ExternalInput") for b in range(NQ)]
    out = nc.dram_tensor("out", [LMAX, 2, F], f32, kind="ExternalOutput")

    # ---- internal DRAM ----------------------------------------------------
    h_q = [nc.dram_tensor(f"h_q{q}", [QTAB, F], bf16, addr_space="Shared")
           for q in range(NQ)]
    ag_in = [nc.dram_tensor(f"ag_in{q}", [QROWS, F], bf16) for q in range(NQ)]

    rg = [list(range(N_CORES))]

    with tile.TileContext(nc) as tc:
        with (
            tc.tile_pool(name="sb", bufs=1) as sb,
            tc.tile_pool(name="msgp", bufs=8) as msgp,
            tc.tile_pool(name="ohp", bufs=4) as ohp,
            tc.tile_pool(name="aggtp", bufs=2) as aggtp,
            tc.tile_pool(name="psw", bufs=4, space="PSUM") as pswp,
            tc.tile_pool(name="psh", bufs=2, space="PSUM") as pshp,
            tc.tile_pool(name="psr", bufs=2, space="PSUM") as psrp,
        ):
            # persistent SBUF
            HT = sb.tile([F, PER], f32, tag="HT")       # H local, feature-major
            AGG = sb.tile([128, W, F], f32, tag="AGG")  # scaled agg rows
            HROWB = sb.tile([128, W, F], bf16, tag="HROWB")  # Hnew row-major
            headf = sb.tile([2, F], f32, tag="headf")   # rows u,v at full prec
            w_in = sb.tile([F, F], f32, tag="w_in")
            w_l = sb.tile([F, F], f32, tag="w_l")
            w_r = sb.tile([F, F], f32, tag="w_r")
            b_in = sb.tile([F, 1], f32, tag="b_in")
            b_l = sb.tile([F, 1], f32, tag="b_l")
            idn = sb.tile([F, F], f32, tag="idn")
            iot = sb.tile([F, F], f32, tag="iot")
            ivd = sb.tile([F, W], f32, tag="ivd")
            dlsb = sb.tile([F, T_total], f32, tag="dlsb")
            gsb = [sb.tile([128, L[b] // 16], i16, tag=f"g{b}", name=f"g{b}")
                   for b in range(NQ)]

            def emit_ag(q):
                """DMA quarter q's Hnew rows to the bounce and AllGather it
                into quarter table q."""
                wq = slice(q * QW, (q + 1) * QW)
                nc.sync.dma_start(
                    ag_in[q][:, :].rearrange("(w p) f -> p w f", p=128),
                    HROWB[:, wq, :])
                nc.gpsimd.collective_compute(
                    "AllGather", mybir.AluOpType.bypass, replica_groups=rg,
                    ins=[ag_in[q].ap().opt()], outs=[h_q[q].ap().opt()])

            # ---- stage 0: loads -------------------------------------------
            nc.sync.dma_start(w_in[:], winT[:, :])
            nc.sync.dma_start(w_l[:], wlT[:, :])
            nc.sync.dma_start(w_r[:], wrT[:, :])
            nc.sync.dma_start(b_in[:], bin_[:, :])
            nc.sync.dma_start(b_l[:], bl[:, :])
            nc.sync.dma_start(idn[:], ident[:, :])
            nc.sync.dma_start(iot[:], iotar[:, :])
            nc.sync.dma_start(ivd[:], invdeg[:, :])
            nc.sync.dma_start(dlsb[:], dl_d[:, :])
            for b in range(NQ):
                nc.sync.dma_start(gsb[b][:], gidx_d[b][:, :])

            # xT staged through AGG viewed feature-major [F, PER]
            AGGf = AGG[:].rearrange("p w f -> p (w f)")
            nc.sync.dma_start(AGGf, xT[:, :])

            # H0 = W_in @ xT + b_in (feature-major), then row-major; each
            # quarter's AllGather is issued as soon as its windows are done
            for w in range(W):
                ws = slice(w * F, (w + 1) * F)
                ph = pshp.tile([F, F], f32, tag="psh")
                nc.tensor.matmul(ph[:], lhsT=w_in[:], rhs=AGGf[:, ws],
                                 start=True, stop=True)
                nc.vector.tensor_scalar_add(HT[:, ws], ph[:], b_in[:, 0:1])
                pr = psrp.tile([F, F], f32, tag="psr")
                nc.tensor.transpose(pr[:], HT[:, ws], idn[:])
                nc.vector.tensor_copy(HROWB[:, w, :], pr[:])
                if (w + 1) % QW == 0:
                    emit_ag(w // QW)

            # ---- steps ----------------------------------------------------
            win_runs = {}
            for (w, b, n) in runs:
                win_runs.setdefault(w, []).append((b, n))
            ntw_max = max(sum(n for (_, n) in rr) for rr in win_runs.values())

            for k in range(LMAX):
                last = k == LMAX - 1
                msg_tiles = {}

                def ensure_chunk(b, j0, k=k, msg_tiles=msg_tiles):
                    if (b, j0) in msg_tiles:
                        return msg_tiles[(b, j0)]
                    n = min(CHUNK, L[b] - j0)
                    msg = msgp.tile([128, CHUNK // 128, F], bf16, tag="msg",
                                    name=f"msg_{k}_{b}_{j0}")
                    cols = slice(j0 // 16, (j0 + n) // 16)
                    nc.gpsimd.dma_gather(
                        out_ap=msg[:, : n // 128, :], in_ap=h_q[b][:, :],
                        idxs_ap=gsb[b][:, cols],
                        num_idxs=n, num_idxs_reg=n, elem_size=F)
                    msg_tiles[(b, j0)] = msg
                    return msg

                # window-major: segment-sum + SAGE update per window, so the
                # update pipeline runs underneath the gather stream
                for w in range(W):
                    ws = slice(w * F, (w + 1) * F)
                    rr = win_runs.get(w, [])
                    if rr:
                        ntw = sum(n for (_, n) in rr)
                        tg0 = tile0[(rr[0][0], w)]
                        oh = ohp.tile([128, ntw_max, F], bf16, tag="oh",
                                      name=f"oh_{k}_{w}")
                        nc.vector.tensor_tensor(
                            out=oh[:, :ntw, :],
                            in0=iot[:].unsqueeze(1).to_broadcast([128, ntw, F]),
                            in1=dlsb[:, tg0:tg0 + ntw].unsqueeze(2)
                                .to_broadcast([128, ntw, F]),
                            op=mybir.AluOpType.is_equal)
                        ps = pswp.tile([128, F], f32, tag="psw")
                        ti = 0
                        for (b, n) in rr:
                            s0 = slot0[(b, w)]
                            for t in range(n):
                                s = s0 + t * F
                                msg = ensure_chunk(b, (s // CHUNK) * CHUNK)
                                nc.tensor.matmul(
                                    ps[:], lhsT=oh[:, ti, :],
                                    rhs=msg[:, (s % CHUNK) // F, :],
                                    start=(ti == 0), stop=(ti == ntw - 1))
                                ti += 1
                        # evacuate with the 1/deg scaling folded in
                        nc.vector.tensor_scalar_mul(AGG[:, w, :], ps[:],
                                                    ivd[:, w:w + 1])
                    else:
                        nc.vector.memset(AGG[:, w, :], 0.0)

                    # Hnew_w = relu(W_l @ aggT + W_r @ HT + b_l)
                    pt = pswp.tile([F, F], f32, tag="psw")
                    nc.tensor.transpose(pt[:], AGG[:, w, :], idn[:])
                    at = aggtp.tile([F, F], f32, tag="aggT")
                    nc.vector.tensor_copy(at[:], pt[:])
                    ph = pshp.tile([F, F], f32, tag="psh")
                    nc.tensor.matmul(ph[:], lhsT=w_l[:], rhs=at[:],
                                     start=True, stop=False)
                    nc.tensor.matmul(ph[:], lhsT=w_r[:], rhs=HT[:, ws],
                                     start=False, stop=True)
                    nc.scalar.activation(HT[:, ws], ph[:],
                                         mybir.ActivationFunctionType.Relu,
                                         bias=b_l[:, 0:1])
                    if not last or w == 0:
                        pr = psrp.tile([F, F], f32, tag="psr")
                        nc.tensor.transpose(pr[:], HT[:, ws], idn[:])
                        if w == 0:
                            nc.vector.tensor_copy(headf[:], pr[0:2, :])
                        if not last:
                            nc.vector.tensor_copy(HROWB[:, w, :], pr[:])
                    if not last and (w + 1) % QW == 0:
                        emit_ag(w // QW)
                # head rows (global rows 0,1 live on core 0, window 0)
                nc.sync.dma_start(out[k, :, :], headf[:])

    # Align each gather's SWDGE queue with the DMASW sem lane Tile assigned
    # (a sem lane must only ever be updated from one queue).
    import re
    for blk in nc.m.functions[0].blocks:
        for ins in blk.instructions:
            if isinstance(ins, mybir.InstDMAGatherAnt) and ins.sync_info:
                m = re.match(r"DMASW(\d+)", ins.sync_info.on_update[0].ant_name)
                if m:
                    ins.queue_num = int(m.group(1)) % 4

    nc.compile()
    return nc


def _heads(out_rows, W_e1, b_e1, W_e2, b_e2, W_h1, b_h1, W_h2, b_h2):
    """Host-side tiny MLP heads, mirroring the reference math in f32."""
    relu = lambda x: np.maximum(x, 0.0)
    alphas, scores = [], []
    p_not = np.float32(1.0)
    for k in range(LMAX):
        h_u = out_rows[k, 0].astype(np.float32)
        h_v = out_rows[k, 1].astype(np.float32)
        feat = np.concatenate([h_u, h_v, h_u * h_v])
        score = relu(feat @ W_e1.T + b_e1) @ W_e2.T + b_e2
        hin = np.concatenate([h_u, h_v, score])
        z = relu(hin @ W_h1.T + b_h1) @ W_h2.T + b_h2
        p_halt = np.float32(1.0) / (np.float32(1.0) + np.exp(-z[0]))
        alphas.append(p_halt * p_not)
        scores.append(score[0])
        p_not = p_not * (np.float32(1.0) - p_halt)
    alpha = np.stack(alphas).astype(np.float32)
    alpha = alpha / (alpha.sum() + np.float32(1e-8))
    scores_v = np.stack(scores).astype(np.float32)
    final_score = (alpha * scores_v).sum()
    depths = np.arange(1, LMAX + 1, dtype=np.float32)
    expected_depth = (alpha * depths).sum()
    return np.float32(final_score), np.float32(expected_depth), alpha


def _make_in_maps(inputs, x_sub, inv_deg, gidx, dls):
    W_in = np.asarray(inputs["W_in"], np.float32)
    W_l = np.asarray(inputs["W_l"], np.float32)
    W_r = np.asarray(inputs["W_r"], np.float32)
    common = dict(
        winT=np.ascontiguousarray(W_in.T),
        wlT=np.ascontiguousarray(W_l.T),
        wrT=np.ascontiguousarray(W_r.T),
        bin=np.asarray(inputs["b_in"], np.float32).reshape(F, 1),
        bl=np.asarray(inputs["b_l"], np.float32).reshape(F, 1),
        ident=np.eye(F, dtype=np.float32),
        iotar=np.tile(np.arange(F, dtype=np.float32), (F, 1)),
    )
    in_maps = []
    for c in range(N_CORES):
        rows = slice(c * PER, (c + 1) * PER)
        m = dict(common)
        m["xT"] = np.ascontiguousarray(x_sub[rows].T)
        m["invdeg"] = np.ascontiguousarray(inv_deg[rows].reshape(W, 128).T)
        m["dl"] = dls[c]
        for b in range(NQ):
            m[f"gidx{b}"] = gidx[c][b]
        in_maps.append(m)
    return in_maps


def _run(inputs, trace=False):
    x_full = np.asarray(inputs["x_full"], np.float32)
    subset = np.asarray(inputs["subset"], np.int64)
    ei = np.asarray(inputs["edge_index"], np.int64)
    src, dst = ei[0], ei[1]

    x_sub = np.zeros((N_PAD, F), np.float32)
    x_sub[:N_SUB] = x_full[subset]
    deg = np.maximum(np.bincount(dst, minlength=N_SUB).astype(np.float32), 1.0)
    inv_deg = np.ones(N_PAD, np.float32)
    inv_deg[:N_SUB] = 1.0 / deg

    gidx, dls, meta = _prep_edges(src, dst)
    nc = _build_graph(meta)
    in_maps = _make_in_maps(inputs, x_sub, inv_deg, gidx, dls)

    res = run_bass_kernel_spmd(nc, in_maps, list(range(N_CORES)), trace=trace)
    out_rows = np.asarray(res.results[0]["out"]).reshape(LMAX, 2, F)

    fs, ed, alpha = _heads(
        out_rows,
        np.asarray(inputs["W_e1"], np.float32), np.asarray(inputs["b_e1"], np.float32),
        np.asarray(inputs["W_e2"], np.float32), np.asarray(inputs["b_e2"], np.float32),
        np.asarray(inputs["W_h1"], np.float32), np.asarray(inputs["b_h1"], np.float32),
        np.asarray(inputs["W_h2"], np.float32), np.asarray(inputs["b_h2"], np.float32),
    )
    return (fs, ed, alpha), res


def kernel(**inputs):
    (fs, ed, alpha), _ = _run(inputs, trace=False)
    return fs, ed, alpha


# revision 20
# speedup vs baseline: 1.0981x; 1.0981x over previous
"""AdaptiveSAGE GNN message-passing kernel for 8 Trainium2 NeuronCores.

Distribution strategy (dst-sharded message passing, PE-based segment sum):
  - Subgraph nodes padded to N_PAD = 81920 = 8 * 10240; core c owns rows
    [c*10240, (c+1)*10240).
  - The replicated H table is split into 4 quarter-tables of 20480 rows
    (so dma_gather's int16 indices always fit); a host-side node
    permutation maps core c's local rows [q*2560, (q+1)*2560) to quarter
    q at offset c*2560, which makes each quarter exactly one AllGather
    whose input is ready as soon as a quarter of the windows is computed
    -- three of the four AllGathers hide under compute.
  - Edges are assigned to the core owning their destination, bucketed by
    (src quarter, dst window of 128 rows), padded per bucket to a multiple
    of 128 so all cores run one identical instruction stream.
  - Messages are fetched with dma_gather in 1024-index chunks spread over
    the 4 SWDGE queues (all gpsimd Q7 pairs generate descriptors).
  - Segment-sum by destination runs on the TensorEngine: per 128-message
    tile a one-hot(dst) matrix (built by a batched DVE is_equal against an
    iota row) is the stationary matmul operand; one PSUM tile accumulates
    a whole window across all quarters (exact, no RMW races).
  - Messages / one-hots / H table are bf16 (halves gather and AllGather
    bytes, enables fast weight load); PSUM, weights, the local H, and the
    u,v head rows stay f32.
  - The tiny MLP heads (scores / halting probs on rows u=0, v=1) are
    evaluated on the host from the 5 x 2 x 128 head rows the kernel emits.
"""

import numpy as np

import concourse.bass as bass
import concourse.bacc as bacc
import concourse.tile as tile
import concourse.mybir as mybir
from concourse.bass_utils import run_bass_kernel_spmd

F = 128          # feature dim
N_CORES = 8
N_SUB = 80000
PER = 10240      # rows per core
N_PAD = N_CORES * PER
W = PER // F     # dst windows of 128 rows per core
NQ = 4           # quarter tables
QROWS = PER // NQ            # rows per core per quarter
QTAB = N_CORES * QROWS       # rows per quarter table (<= 32767 for int16)
QW = W // NQ                 # windows per quarter
LMAX = 5
CHUNK = 1024     # messages per gather chunk (SWDGE ring caps num_idxs ~<2K)
PAD_DL = 999.0   # out-of-window dst marker for padding slots


def _wrap16(idx: np.ndarray) -> np.ndarray:
    """SWDGE index layout: logical i -> [i%16, i//16], replicated across the
    8 groups of 16 partitions."""
    n = idx.shape[0]
    assert n % 16 == 0
    w = idx.reshape(n // 16, 16).T.astype(np.int16)
    return np.tile(w, (8, 1))


def _prep_edges(src: np.ndarray, dst: np.ndarray):
    """Bucket edges by (dst core, src quarter, dst window); pad each bucket
    to a common multiple-of-128 so the SPMD graph is uniform across cores.

    Gather indices address the permuted quarter tables: node id g with
    c = g // PER, r = g % PER lives in quarter r // QROWS at row
    c * QROWS + (r % QROWS).
    """
    assert QTAB <= 32768
    nw = W
    core_of = dst // PER
    src_c = src // PER
    src_r = src % PER
    bank_of = src_r // QROWS                    # quarter table
    src_idx = src_c * QROWS + (src_r % QROWS)   # row within quarter table
    dst_local = dst - core_of * PER
    w_of = dst_local // F
    run_of = bank_of * nw + w_of

    counts = np.zeros((N_CORES, NQ * nw), dtype=np.int64)
    per_core = []
    for c in range(N_CORES):
        m = core_of == c
        gl = src_idx[m].astype(np.int16)
        dl = (dst_local[m] % F).astype(np.float32)
        rid = run_of[m]
        order = np.argsort(rid, kind="stable")
        gl, dl, rid = gl[order], dl[order], rid[order]
        bounds = np.searchsorted(rid, np.arange(NQ * nw + 1))
        counts[c] = bounds[1:] - bounds[:-1]
        per_core.append((gl, dl, bounds))

    nt = np.ceil(counts.max(axis=0) / F).astype(np.int64)  # tiles per run
    # window-major run order (w, b): a window's bank runs are consecutive,
    # so one PSUM tile accumulates them all
    runs = [(w, b, int(nt[b * nw + w]))
            for w in range(nw) for b in range(NQ) if nt[b * nw + w] > 0]
    # per-bank padded slot counts (bank-major gather layout, window-sorted
    # within each bank) and per-run tile offsets in (w, b) order
    L = [0] * NQ
    slot0 = {}
    for b in range(NQ):
        for w in range(nw):
            n = int(nt[b * nw + w])
            if n:
                slot0[(b, w)] = L[b]
                L[b] += n * F
    tile0 = {}
    tg = 0
    for (w, b, n) in runs:
        tile0[(b, w)] = tg
        tg += n
    T_total = tg

    gidx, dls = [], []
    for c in range(N_CORES):
        gl, dl, bounds = per_core[c]
        gb = [np.zeros(L[b], np.int16) for b in range(NQ)]
        dla = np.full(T_total * F, PAD_DL, np.float32)
        for (w, b, n) in runs:
            r = b * nw + w
            seg = slice(bounds[r], bounds[r + 1])
            cnt = bounds[r + 1] - bounds[r]
            s0 = slot0[(b, w)]
            gb[b][s0:s0 + cnt] = gl[seg]
            t0 = tile0[(b, w)]
            dla[t0 * F:t0 * F + cnt] = dl[seg]
        gidx.append([_wrap16(x) for x in gb])
        dls.append(np.ascontiguousarray(dla.reshape(T_total, F).T))
    meta = dict(L=L, runs=runs, T_total=T_total, slot0=slot0, tile0=tile0)
    return gidx, dls, meta


def _build_graph(meta):
    """Build the SPMD Bass graph (identical for all 8 cores)."""
    L = meta["L"]
    runs = meta["runs"]
    T_total = meta["T_total"]
    slot0 = meta["slot0"]
    tile0 = meta["tile0"]
    f32 = mybir.dt.float32
    bf16 = mybir.dt.bfloat16
    i16 = mybir.dt.int16
    nc = bacc.Bacc("TRN2", target_bir_lowering=False, debug=False,
                   num_devices=N_CORES, num_swdge_queues=4)

    # ---- kernel I/O -------------------------------------------------------
    xT = nc.dram_tensor("xT", [F, PER], f32, kind="ExternalInput")
    invdeg = nc.dram_tensor("invdeg", [F, W], f32, kind="ExternalInput")
    winT = nc.dram_tensor("winT", [F, F], f32, kind="ExternalInput")
    wlT = nc.dram_tensor("wlT", [F, F], f32, kind="ExternalInput")
    wrT = nc.dram_tensor("wrT", [F, F], f32, kind="ExternalInput")
    bin_ = nc.dram_tensor("bin", [F, 1], f32, kind="ExternalInput")
    bl = nc.dram_tensor("bl", [F, 1], f32, kind="ExternalInput")
    ident = nc.dram_tensor("ident", [F, F], f32, kind="ExternalInput")
    iotar = nc.dram_tensor("iotar", [F, F], f32, kind="ExternalInput")
    dl_d = nc.dram_tensor("dl", [F, T_total], f32, kind="ExternalInput")
    gidx_d = [nc.dram_tensor(f"gidx{b}", [128, L[b] // 16], i16,
                             kind="ExternalInput") for b in range(NQ)]
    out = nc.dram_tensor("out", [LMAX, 2, F], f32, kind="ExternalOutput")

    # ---- internal DRAM ----------------------------------------------------
    h_q = [[nc.dram_tensor(f"h_q{p}_{q}", [QTAB, F], bf16,
                           addr_space="Shared") for q in range(NQ)]
           for p in range(2)]
    ag_in = [nc.dram_tensor(f"ag_in{q}", [QROWS, F], bf16) for q in range(NQ)]

    rg = [list(range(N_CORES))]

    with tile.TileContext(nc) as tc:
        with (
            tc.tile_pool(name="sb", bufs=1) as sb,
            tc.tile_pool(name="msgp", bufs=8) as msgp,
            tc.tile_pool(name="ohp", bufs=4) as ohp,
            tc.tile_pool(name="aggtp", bufs=2) as aggtp,
            tc.tile_pool(name="psw", bufs=4, space="PSUM") as pswp,
            tc.tile_pool(name="psh", bufs=2, space="PSUM") as pshp,
            tc.tile_pool(name="psr", bufs=2, space="PSUM") as psrp,
        ):
            # persistent SBUF
            HT = sb.tile([F, PER], f32, tag="HT")       # H local, feature-major
            AGG = sb.tile([128, W, F], f32, tag="AGG")  # scaled agg rows
            HROWB = sb.tile([128, W, F], bf16, tag="HROWB")  # Hnew row-major
            headf = sb.tile([2, F], f32, tag="headf")   # rows u,v at full prec
            w_in = sb.tile([F, F], f32, tag="w_in")
            w_l = sb.tile([F, F], f32, tag="w_l")
            w_r = sb.tile([F, F], f32, tag="w_r")
            b_in = sb.tile([F, 1], f32, tag="b_in")
            b_l = sb.tile([F, 1], f32, tag="b_l")
            idn = sb.tile([F, F], f32, tag="idn")
            iot = sb.tile([F, F], f32, tag="iot")
            ivd = sb.tile([F, W], f32, tag="ivd")
            dlsb = sb.tile([F, T_total], f32, tag="dlsb")
            gsb = [sb.tile([128, L[b] // 16], i16, tag=f"g{b}", name=f"g{b}")
                   for b in range(NQ)]

            def emit_ag(p, q):
                """DMA quarter q's Hnew rows to the bounce and AllGather it
                into quarter table q of table set p."""
                wq = slice(q * QW, (q + 1) * QW)
                nc.sync.dma_start(
                    ag_in[q][:, :].rearrange("(w p) f -> p w f", p=128),
                    HROWB[:, wq, :])
                nc.gpsimd.collective_compute(
                    "AllGather", mybir.AluOpType.bypass, replica_groups=rg,
                    ins=[ag_in[q].ap().opt()], outs=[h_q[p][q].ap().opt()])

            # ---- stage 0: loads -------------------------------------------
            nc.sync.dma_start(w_in[:], winT[:, :])
            nc.sync.dma_start(w_l[:], wlT[:, :])
            nc.sync.dma_start(w_r[:], wrT[:, :])
            nc.sync.dma_start(b_in[:], bin_[:, :])
            nc.sync.dma_start(b_l[:], bl[:, :])
            nc.sync.dma_start(idn[:], ident[:, :])
            nc.sync.dma_start(iot[:], iotar[:, :])
            nc.sync.dma_start(ivd[:], invdeg[:, :])
            nc.sync.dma_start(dlsb[:], dl_d[:, :])
            for b in range(NQ):
                nc.sync.dma_start(gsb[b][:], gidx_d[b][:, :])

            # xT staged through AGG viewed feature-major [F, PER]
            AGGf = AGG[:].rearrange("p w f -> p (w f)")
            nc.sync.dma_start(AGGf, xT[:, :])

            # H0 = W_in @ xT + b_in (feature-major), then row-major; each
            # quarter's AllGather is issued as soon as its windows are done
            for w in range(W):
                ws = slice(w * F, (w + 1) * F)
                ph = pshp.tile([F, F], f32, tag="psh")
                nc.tensor.matmul(ph[:], lhsT=w_in[:], rhs=AGGf[:, ws],
                                 start=True, stop=True)
                nc.vector.tensor_scalar_add(HT[:, ws], ph[:], b_in[:, 0:1])
                pr = psrp.tile([F, F], f32, tag="psr")
                nc.tensor.transpose(pr[:], HT[:, ws], idn[:])
                nc.vector.tensor_copy(HROWB[:, w, :], pr[:])
                if (w + 1) % QW == 0:
                    emit_ag(0, w // QW)

            # ---- steps ----------------------------------------------------
            win_runs = {}
            for (w, b, n) in runs:
                win_runs.setdefault(w, []).append((b, n))
            ntw_max = max(sum(n for (_, n) in rr) for rr in win_runs.values())

            for k in range(LMAX):
                last = k == LMAX - 1
                msg_tiles = {}

                def ensure_chunk(b, j0, k=k, msg_tiles=msg_tiles):
                    if (b, j0) in msg_tiles:
                        return msg_tiles[(b, j0)]
                    n = min(CHUNK, L[b] - j0)
                    msg = msgp.tile([128, CHUNK // 128, F], bf16, tag="msg",
                                    name=f"msg_{k}_{b}_{j0}")
                    cols = slice(j0 // 16, (j0 + n) // 16)
                    nc.gpsimd.dma_gather(
                        out_ap=msg[:, : n // 128, :],
                        in_ap=h_q[k % 2][b][:, :],
                        idxs_ap=gsb[b][:, cols],
                        num_idxs=n, num_idxs_reg=n, elem_size=F)
                    msg_tiles[(b, j0)] = msg
                    return msg

                # window-major: segment-sum + SAGE update per window, so the
                # update pipeline runs underneath the gather stream
                for w in range(W):
                    ws = slice(w * F, (w + 1) * F)
                    rr = win_runs.get(w, [])
                    if rr:
                        ntw = sum(n for (_, n) in rr)
                        tg0 = tile0[(rr[0][0], w)]
                        oh = ohp.tile([128, ntw_max, F], bf16, tag="oh",
                                      name=f"oh_{k}_{w}")
                        nc.vector.tensor_tensor(
                            out=oh[:, :ntw, :],
                            in0=iot[:].unsqueeze(1).to_broadcast([128, ntw, F]),
                            in1=dlsb[:, tg0:tg0 + ntw].unsqueeze(2)
                                .to_broadcast([128, ntw, F]),
                            op=mybir.AluOpType.is_equal)
                        ps = pswp.tile([128, F], f32, tag="psw")
                        ti = 0
                        for (b, n) in rr:
                            s0 = slot0[(b, w)]
                            for t in range(n):
                                s = s0 + t * F
                                msg = ensure_chunk(b, (s // CHUNK) * CHUNK)
                                nc.tensor.matmul(
                                    ps[:], lhsT=oh[:, ti, :],
                                    rhs=msg[:, (s % CHUNK) // F, :],
                                    start=(ti == 0), stop=(ti == ntw - 1))
                                ti += 1
                        # evacuate with the 1/deg scaling folded in
                        nc.vector.tensor_scalar_mul(AGG[:, w, :], ps[:],
                                                    ivd[:, w:w + 1])
                    else:
                        nc.vector.memset(AGG[:, w, :], 0.0)

                    # Hnew_w = relu(W_l @ aggT + W_r @ HT + b_l)
                    pt = pswp.tile([F, F], f32, tag="psw")
                    nc.tensor.transpose(pt[:], AGG[:, w, :], idn[:])
                    at = aggtp.tile([F, F], f32, tag="aggT")
                    nc.vector.tensor_copy(at[:], pt[:])
                    ph = pshp.tile([F, F], f32, tag="psh")
                    nc.tensor.matmul(ph[:], lhsT=w_l[:], rhs=at[:],
                                     start=True, stop=False)
                    nc.tensor.matmul(ph[:], lhsT=w_r[:], rhs=HT[:, ws],
                                     start=False, stop=True)
                    nc.scalar.activation(HT[:, ws], ph[:],
                                         mybir.ActivationFunctionType.Relu,
                                         bias=b_l[:, 0:1])
                    if not last or w == 0:
                        pr = psrp.tile([F, F], f32, tag="psr")
                        nc.tensor.transpose(pr[:], HT[:, ws], idn[:])
                        if w == 0:
                            nc.vector.tensor_copy(headf[:], pr[0:2, :])
                        if not last:
                            nc.vector.tensor_copy(HROWB[:, w, :], pr[:])
                    if not last and (w + 1) % QW == 0:
                        emit_ag((k + 1) % 2, w // QW)
                # head rows (global rows 0,1 live on core 0, window 0)
                nc.sync.dma_start(out[k, :, :], headf[:])

    # Align each gather's SWDGE queue with the DMASW sem lane Tile assigned
    # (a sem lane must only ever be updated from one queue).
    import re
    for blk in nc.m.functions[0].blocks:
        for ins in blk.instructions:
            if isinstance(ins, mybir.InstDMAGatherAnt) and ins.sync_info:
                m = re.match(r"DMASW(\d+)", ins.sync_info.on_update[0].ant_name)
                if m:
                    ins.queue_num = int(m.group(1)) % 4

    nc.compile()
    return nc


def _heads(out_rows, W_e1, b_e1, W_e2, b_e2, W_h1, b_h1, W_h2, b_h2):
    """Host-side tiny MLP heads, mirroring the reference math in f32."""
    relu = lambda x: np.maximum(x, 0.0)
    alphas, scores = [], []
    p_not = np.float32(1.0)
    for k in range(LMAX):
        h_u = out_rows[k, 0].astype(np.float32)
        h_v = out_rows[k, 1].astype(np.float32)
        feat = np.concatenate([h_u, h_v, h_u * h_v])
        score = relu(feat @ W_e1.T + b_e1) @ W_e2.T + b_e2
        hin = np.concatenate([h_u, h_v, score])
        z = relu(hin @ W_h1.T + b_h1) @ W_h2.T + b_h2
        p_halt = np.float32(1.0) / (np.float32(1.0) + np.exp(-z[0]))
        alphas.append(p_halt * p_not)
        scores.append(score[0])
        p_not = p_not * (np.float32(1.0) - p_halt)
    alpha = np.stack(alphas).astype(np.float32)
    alpha = alpha / (alpha.sum() + np.float32(1e-8))
    scores_v = np.stack(scores).astype(np.float32)
    final_score = (alpha * scores_v).sum()
    depths = np.arange(1, LMAX + 1, dtype=np.float32)
    expected_depth = (alpha * depths).sum()
    return np.float32(final_score), np.float32(expected_depth), alpha


def _make_in_maps(inputs, x_sub, inv_deg, gidx, dls):
    W_in = np.asarray(inputs["W_in"], np.float32)
    W_l = np.asarray(inputs["W_l"], np.float32)
    W_r = np.asarray(inputs["W_r"], np.float32)
    common = dict(
        winT=np.ascontiguousarray(W_in.T),
        wlT=np.ascontiguousarray(W_l.T),
        wrT=np.ascontiguousarray(W_r.T),
        bin=np.asarray(inputs["b_in"], np.float32).reshape(F, 1),
        bl=np.asarray(inputs["b_l"], np.float32).reshape(F, 1),
        ident=np.eye(F, dtype=np.float32),
        iotar=np.tile(np.arange(F, dtype=np.float32), (F, 1)),
    )
    in_maps = []
    for c in range(N_CORES):
        rows = slice(c * PER, (c + 1) * PER)
        m = dict(common)
        m["xT"] = np.ascontiguousarray(x_sub[rows].T)
        m["invdeg"] = np.ascontiguousarray(inv_deg[rows].reshape(W, 128).T)
        m["dl"] = dls[c]
        for b in range(NQ):
            m[f"gidx{b}"] = gidx[c][b]
        in_maps.append(m)
    return in_maps


def _run(inputs, trace=False):
    x_full = np.asarray(inputs["x_full"], np.float32)
    subset = np.asarray(inputs["subset"], np.int64)
    ei = np.asarray(inputs["edge_index"], np.int64)
    src, dst = ei[0], ei[1]

    x_sub = np.zeros((N_PAD, F), np.float32)
    x_sub[:N_SUB] = x_full[subset]
    deg = np.maximum(np.bincount(dst, minlength=N_SUB).astype(np.float32), 1.0)
    inv_deg = np.ones(N_PAD, np.float32)
    inv_deg[:N_SUB] = 1.0 / deg

    gidx, dls, meta = _prep_edges(src, dst)
    nc = _build_graph(meta)
    in_maps = _make_in_maps(inputs, x_sub, inv_deg, gidx, dls)

    res = run_bass_kernel_spmd(nc, in_maps, list(range(N_CORES)), trace=trace)
    out_rows = np.asarray(res.results[0]["out"]).reshape(LMAX, 2, F)

    fs, ed, alpha = _heads(
        out_rows,
        np.asarray(inputs["W_e1"], np.float32), np.asarray(inputs["b_e1"], np.float32),
        np.asarray(inputs["W_e2"], np.float32), np.asarray(inputs["b_e2"], np.float32),
        np.asarray(inputs["W_h1"], np.float32), np.asarray(inputs["b_h1"], np.float32),
        np.asarray(inputs["W_h2"], np.float32), np.asarray(inputs["b_h2"], np.float32),
    )
    return (fs, ed, alpha), res


def kernel(**inputs):
    (fs, ed, alpha), _ = _run(inputs, trace=False)
    return fs, ed, alpha


# revision 21
# speedup vs baseline: 1.1949x; 1.0882x over previous
"""AdaptiveSAGE GNN message-passing kernel for 8 Trainium2 NeuronCores.

Distribution strategy (dst-sharded message passing, PE-based segment sum):
  - Subgraph nodes padded to N_PAD = 81920 = 8 * 10240; core c owns rows
    [c*10240, (c+1)*10240).
  - The replicated H table is split into 4 quarter-tables of 20480 rows
    (so dma_gather's int16 indices always fit); a host-side node
    permutation maps core c's local rows [q*2560, (q+1)*2560) to quarter
    q at offset c*2560, which makes each quarter exactly one AllGather
    whose input is ready as soon as a quarter of the windows is computed
    -- three of the four AllGathers hide under compute.
  - Edges are assigned to the core owning their destination, bucketed by
    (src quarter, dst window of 128 rows), padded per bucket to a multiple
    of 128 so all cores run one identical instruction stream.
  - Messages are fetched with dma_gather in 1024-index chunks spread over
    the 4 SWDGE queues (all gpsimd Q7 pairs generate descriptors).
  - Segment-sum by destination runs on the TensorEngine: per 128-message
    tile a one-hot(dst) matrix (built by a batched DVE is_equal against an
    iota row) is the stationary matmul operand; one PSUM tile accumulates
    a whole window across all quarters (exact, no RMW races).
  - Messages / one-hots / H table are bf16 (halves gather and AllGather
    bytes, enables fast weight load); PSUM, weights, the local H, and the
    u,v head rows stay f32.
  - The tiny MLP heads (scores / halting probs on rows u=0, v=1) are
    evaluated on the host from the 5 x 2 x 128 head rows the kernel emits.
"""

import numpy as np

import concourse.bass as bass
import concourse.bacc as bacc
import concourse.tile as tile
import concourse.mybir as mybir
from concourse.bass_utils import run_bass_kernel_spmd

F = 128          # feature dim
N_CORES = 8
N_SUB = 80000
PER = 10240      # rows per core
N_PAD = N_CORES * PER
W = PER // F     # dst windows of 128 rows per core
NQ = 4           # quarter tables
QROWS = PER // NQ            # rows per core per quarter
QTAB = N_CORES * QROWS       # rows per quarter table (<= 32767 for int16)
QW = W // NQ                 # windows per quarter
LMAX = 5
CHUNK = 1024     # messages per gather chunk (SWDGE ring caps num_idxs ~<2K)
PAD_DL = 999.0   # out-of-window dst marker for padding slots


def _wrap16(idx: np.ndarray) -> np.ndarray:
    """SWDGE index layout: logical i -> [i%16, i//16], replicated across the
    8 groups of 16 partitions."""
    n = idx.shape[0]
    assert n % 16 == 0
    w = idx.reshape(n // 16, 16).T.astype(np.int16)
    return np.tile(w, (8, 1))


def _prep_edges(src: np.ndarray, dst: np.ndarray):
    """Bucket edges by (dst core, src quarter, dst window); pad each bucket
    to a common multiple-of-128 so the SPMD graph is uniform across cores.

    Gather indices address the permuted quarter tables: node id g with
    c = g // PER, r = g % PER lives in quarter r // QROWS at row
    c * QROWS + (r % QROWS).
    """
    assert QTAB <= 32768
    nw = W
    core_of = dst // PER
    src_c = src // PER
    src_r = src % PER
    bank_of = src_r // QROWS                    # quarter table
    src_idx = src_c * QROWS + (src_r % QROWS)   # row within quarter table
    dst_local = dst - core_of * PER
    w_of = dst_local // F
    run_of = bank_of * nw + w_of

    counts = np.zeros((N_CORES, NQ * nw), dtype=np.int64)
    per_core = []
    for c in range(N_CORES):
        m = core_of == c
        gl = src_idx[m].astype(np.int16)
        dl = (dst_local[m] % F).astype(np.float32)
        rid = run_of[m]
        order = np.argsort(rid, kind="stable")
        gl, dl, rid = gl[order], dl[order], rid[order]
        bounds = np.searchsorted(rid, np.arange(NQ * nw + 1))
        counts[c] = bounds[1:] - bounds[:-1]
        per_core.append((gl, dl, bounds))

    nt = np.ceil(counts.max(axis=0) / F).astype(np.int64)  # tiles per run
    # window-major run order (w, b): a window's bank runs are consecutive,
    # so one PSUM tile accumulates them all
    runs = [(w, b, int(nt[b * nw + w]))
            for w in range(nw) for b in range(NQ) if nt[b * nw + w] > 0]
    # per-bank padded slot counts (bank-major gather layout, window-sorted
    # within each bank) and per-run tile offsets in (w, b) order
    L = [0] * NQ
    slot0 = {}
    for b in range(NQ):
        for w in range(nw):
            n = int(nt[b * nw + w])
            if n:
                slot0[(b, w)] = L[b]
                L[b] += n * F
    tile0 = {}
    tg = 0
    for (w, b, n) in runs:
        tile0[(b, w)] = tg
        tg += n
    T_total = tg

    gidx, dls = [], []
    for c in range(N_CORES):
        gl, dl, bounds = per_core[c]
        gb = [np.zeros(L[b], np.int16) for b in range(NQ)]
        dla = np.full(T_total * F, PAD_DL, np.float32)
        for (w, b, n) in runs:
            r = b * nw + w
            seg = slice(bounds[r], bounds[r + 1])
            cnt = bounds[r + 1] - bounds[r]
            s0 = slot0[(b, w)]
            gb[b][s0:s0 + cnt] = gl[seg]
            t0 = tile0[(b, w)]
            dla[t0 * F:t0 * F + cnt] = dl[seg]
        gidx.append([_wrap16(x) for x in gb])
        dls.append(np.ascontiguousarray(dla.reshape(T_total, F).T))
    meta = dict(L=L, runs=runs, T_total=T_total, slot0=slot0, tile0=tile0)
    return gidx, dls, meta


def _build_graph(meta):
    """Build the SPMD Bass graph (identical for all 8 cores)."""
    L = meta["L"]
    runs = meta["runs"]
    T_total = meta["T_total"]
    slot0 = meta["slot0"]
    tile0 = meta["tile0"]
    f32 = mybir.dt.float32
    bf16 = mybir.dt.bfloat16
    i16 = mybir.dt.int16
    nc = bacc.Bacc("TRN2", target_bir_lowering=False, debug=False,
                   num_devices=N_CORES, num_swdge_queues=4)

    # ---- kernel I/O -------------------------------------------------------
    xT = nc.dram_tensor("xT", [F, PER], f32, kind="ExternalInput")
    invdeg = nc.dram_tensor("invdeg", [F, W], f32, kind="ExternalInput")
    winT = nc.dram_tensor("winT", [F, F], f32, kind="ExternalInput")
    wlT = nc.dram_tensor("wlT", [F, F], f32, kind="ExternalInput")
    wrT = nc.dram_tensor("wrT", [F, F], f32, kind="ExternalInput")
    bin_ = nc.dram_tensor("bin", [F, 1], f32, kind="ExternalInput")
    bl = nc.dram_tensor("bl", [F, 1], f32, kind="ExternalInput")
    ident = nc.dram_tensor("ident", [F, F], f32, kind="ExternalInput")
    iotar = nc.dram_tensor("iotar", [F, F], f32, kind="ExternalInput")
    dl_d = nc.dram_tensor("dl", [F, T_total], f32, kind="ExternalInput")
    gidx_d = [nc.dram_tensor(f"gidx{b}", [128, L[b] // 16], i16,
                             kind="ExternalInput") for b in range(NQ)]
    out = nc.dram_tensor("out", [LMAX, 2, F], f32, kind="ExternalOutput")

    # ---- internal DRAM ----------------------------------------------------
    h_q = [[nc.dram_tensor(f"h_q{p}_{q}", [QTAB, F], bf16,
                           addr_space="Shared") for q in range(NQ)]
           for p in range(2)]
    ag_in = [nc.dram_tensor(f"ag_in{q}", [QROWS, F], bf16) for q in range(NQ)]

    rg = [list(range(N_CORES))]

    with tile.TileContext(nc) as tc:
        with (
            tc.tile_pool(name="sb", bufs=1) as sb,
            tc.tile_pool(name="msgp", bufs=8) as msgp,
            tc.tile_pool(name="ohp", bufs=4) as ohp,
            tc.tile_pool(name="aggtp", bufs=2) as aggtp,
            tc.tile_pool(name="psw", bufs=4, space="PSUM") as pswp,
            tc.tile_pool(name="psh", bufs=2, space="PSUM") as pshp,
            tc.tile_pool(name="psr", bufs=2, space="PSUM") as psrp,
        ):
            # persistent SBUF
            HT = sb.tile([F, PER], f32, tag="HT")       # H local, feature-major
            AGG = sb.tile([128, W, F], f32, tag="AGG")  # scaled agg rows
            HROWB = sb.tile([128, W, F], bf16, tag="HROWB")  # Hnew row-major
            headf = sb.tile([2, F], f32, tag="headf")   # rows u,v at full prec
            w_in = sb.tile([F, F], f32, tag="w_in")
            w_l = sb.tile([F, F], f32, tag="w_l")
            w_r = sb.tile([F, F], f32, tag="w_r")
            b_in = sb.tile([F, 1], f32, tag="b_in")
            b_l = sb.tile([F, 1], f32, tag="b_l")
            idn = sb.tile([F, F], f32, tag="idn")
            iot = sb.tile([F, F], f32, tag="iot")
            ivd = sb.tile([F, W], f32, tag="ivd")
            dlsb = sb.tile([F, T_total], f32, tag="dlsb")
            gsb = [sb.tile([128, L[b] // 16], i16, tag=f"g{b}", name=f"g{b}")
                   for b in range(NQ)]

            def emit_ag(p, q):
                """DMA quarter q's Hnew rows to the bounce and AllGather it
                into quarter table q of table set p."""
                wq = slice(q * QW, (q + 1) * QW)
                nc.sync.dma_start(
                    ag_in[q][:, :].rearrange("(w p) f -> p w f", p=128),
                    HROWB[:, wq, :])
                nc.gpsimd.collective_compute(
                    "AllGather", mybir.AluOpType.bypass, replica_groups=rg,
                    ins=[ag_in[q].ap().opt()], outs=[h_q[p][q].ap().opt()])

            # ---- stage 0: loads -------------------------------------------
            nc.sync.dma_start(w_in[:], winT[:, :])
            nc.sync.dma_start(w_l[:], wlT[:, :])
            nc.sync.dma_start(w_r[:], wrT[:, :])
            nc.sync.dma_start(b_in[:], bin_[:, :])
            nc.sync.dma_start(b_l[:], bl[:, :])
            nc.sync.dma_start(idn[:], ident[:, :])
            nc.sync.dma_start(iot[:], iotar[:, :])
            nc.sync.dma_start(ivd[:], invdeg[:, :])
            nc.sync.dma_start(dlsb[:], dl_d[:, :])
            for b in range(NQ):
                nc.sync.dma_start(gsb[b][:], gidx_d[b][:, :])

            # xT staged through AGG viewed feature-major [F, PER]
            AGGf = AGG[:].rearrange("p w f -> p (w f)")
            nc.sync.dma_start(AGGf, xT[:, :])

            # H0 = W_in @ xT + b_in (feature-major), then row-major; each
            # quarter's AllGather is issued as soon as its windows are done
            for w in range(W):
                ws = slice(w * F, (w + 1) * F)
                ph = pshp.tile([F, F], f32, tag="psh")
                nc.tensor.matmul(ph[:], lhsT=w_in[:], rhs=AGGf[:, ws],
                                 start=True, stop=True)
                nc.vector.tensor_scalar_add(HT[:, ws], ph[:], b_in[:, 0:1])
                pr = psrp.tile([F, F], f32, tag="psr")
                nc.tensor.transpose(pr[:], HT[:, ws], idn[:])
                nc.vector.tensor_copy(HROWB[:, w, :], pr[:])
            for q in range(NQ):
                emit_ag(0, q)

            # ---- steps ----------------------------------------------------
            win_runs = {}
            for (w, b, n) in runs:
                win_runs.setdefault(w, []).append((b, n))
            ntw_max = max(sum(n for (_, n) in rr) for rr in win_runs.values())

            for k in range(LMAX):
                last = k == LMAX - 1
                msg_tiles = {}

                def ensure_chunk(b, j0, k=k, msg_tiles=msg_tiles):
                    if (b, j0) in msg_tiles:
                        return msg_tiles[(b, j0)]
                    n = min(CHUNK, L[b] - j0)
                    msg = msgp.tile([128, CHUNK // 128, F], bf16, tag="msg",
                                    name=f"msg_{k}_{b}_{j0}")
                    cols = slice(j0 // 16, (j0 + n) // 16)
                    nc.gpsimd.dma_gather(
                        out_ap=msg[:, : n // 128, :],
                        in_ap=h_q[k % 2][b][:, :],
                        idxs_ap=gsb[b][:, cols],
                        num_idxs=n, num_idxs_reg=n, elem_size=F)
                    msg_tiles[(b, j0)] = msg
                    return msg

                # window-major: segment-sum + SAGE update per window, so the
                # update pipeline runs underneath the gather stream
                for w in range(W):
                    ws = slice(w * F, (w + 1) * F)
                    rr = win_runs.get(w, [])
                    if rr:
                        ntw = sum(n for (_, n) in rr)
                        tg0 = tile0[(rr[0][0], w)]
                        oh = ohp.tile([128, ntw_max, F], bf16, tag="oh",
                                      name=f"oh_{k}_{w}")
                        nc.vector.tensor_tensor(
                            out=oh[:, :ntw, :],
                            in0=iot[:].unsqueeze(1).to_broadcast([128, ntw, F]),
                            in1=dlsb[:, tg0:tg0 + ntw].unsqueeze(2)
                                .to_broadcast([128, ntw, F]),
                            op=mybir.AluOpType.is_equal)
                        ps = pswp.tile([128, F], f32, tag="psw")
                        ti = 0
                        for (b, n) in rr:
                            s0 = slot0[(b, w)]
                            for t in range(n):
                                s = s0 + t * F
                                msg = ensure_chunk(b, (s // CHUNK) * CHUNK)
                                nc.tensor.matmul(
                                    ps[:], lhsT=oh[:, ti, :],
                                    rhs=msg[:, (s % CHUNK) // F, :],
                                    start=(ti == 0), stop=(ti == ntw - 1))
                                ti += 1
                        # evacuate with the 1/deg scaling folded in
                        nc.vector.tensor_scalar_mul(AGG[:, w, :], ps[:],
                                                    ivd[:, w:w + 1])
                    else:
                        nc.vector.memset(AGG[:, w, :], 0.0)

                    # Hnew_w = relu(W_l @ aggT + W_r @ HT + b_l)
                    pt = pswp.tile([F, F], f32, tag="psw")
                    nc.tensor.transpose(pt[:], AGG[:, w, :], idn[:])
                    at = aggtp.tile([F, F], f32, tag="aggT")
                    nc.vector.tensor_copy(at[:], pt[:])
                    ph = pshp.tile([F, F], f32, tag="psh")
                    nc.tensor.matmul(ph[:], lhsT=w_l[:], rhs=at[:],
                                     start=True, stop=False)
                    nc.tensor.matmul(ph[:], lhsT=w_r[:], rhs=HT[:, ws],
                                     start=False, stop=True)
                    nc.scalar.activation(HT[:, ws], ph[:],
                                         mybir.ActivationFunctionType.Relu,
                                         bias=b_l[:, 0:1])
                    if not last or w == 0:
                        pr = psrp.tile([F, F], f32, tag="psr")
                        nc.tensor.transpose(pr[:], HT[:, ws], idn[:])
                        if w == 0:
                            nc.vector.tensor_copy(headf[:], pr[0:2, :])
                        if not last:
                            nc.vector.tensor_copy(HROWB[:, w, :], pr[:])
                # head rows (global rows 0,1 live on core 0, window 0)
                nc.sync.dma_start(out[k, :, :], headf[:])
                if not last:
                    for q in range(NQ):
                        emit_ag((k + 1) % 2, q)

    # Align each gather's SWDGE queue with the DMASW sem lane Tile assigned
    # (a sem lane must only ever be updated from one queue).
    import re
    for blk in nc.m.functions[0].blocks:
        for ins in blk.instructions:
            if isinstance(ins, mybir.InstDMAGatherAnt) and ins.sync_info:
                m = re.match(r"DMASW(\d+)", ins.sync_info.on_update[0].ant_name)
                if m:
                    ins.queue_num = int(m.group(1)) % 4

    nc.compile()
    return nc


def _heads(out_rows, W_e1, b_e1, W_e2, b_e2, W_h1, b_h1, W_h2, b_h2):
    """Host-side tiny MLP heads, mirroring the reference math in f32."""
    relu = lambda x: np.maximum(x, 0.0)
    alphas, scores = [], []
    p_not = np.float32(1.0)
    for k in range(LMAX):
        h_u = out_rows[k, 0].astype(np.float32)
        h_v = out_rows[k, 1].astype(np.float32)
        feat = np.concatenate([h_u, h_v, h_u * h_v])
        score = relu(feat @ W_e1.T + b_e1) @ W_e2.T + b_e2
        hin = np.concatenate([h_u, h_v, score])
        z = relu(hin @ W_h1.T + b_h1) @ W_h2.T + b_h2
        p_halt = np.float32(1.0) / (np.float32(1.0) + np.exp(-z[0]))
        alphas.append(p_halt * p_not)
        scores.append(score[0])
        p_not = p_not * (np.float32(1.0) - p_halt)
    alpha = np.stack(alphas).astype(np.float32)
    alpha = alpha / (alpha.sum() + np.float32(1e-8))
    scores_v = np.stack(scores).astype(np.float32)
    final_score = (alpha * scores_v).sum()
    depths = np.arange(1, LMAX + 1, dtype=np.float32)
    expected_depth = (alpha * depths).sum()
    return np.float32(final_score), np.float32(expected_depth), alpha


def _make_in_maps(inputs, x_sub, inv_deg, gidx, dls):
    W_in = np.asarray(inputs["W_in"], np.float32)
    W_l = np.asarray(inputs["W_l"], np.float32)
    W_r = np.asarray(inputs["W_r"], np.float32)
    common = dict(
        winT=np.ascontiguousarray(W_in.T),
        wlT=np.ascontiguousarray(W_l.T),
        wrT=np.ascontiguousarray(W_r.T),
        bin=np.asarray(inputs["b_in"], np.float32).reshape(F, 1),
        bl=np.asarray(inputs["b_l"], np.float32).reshape(F, 1),
        ident=np.eye(F, dtype=np.float32),
        iotar=np.tile(np.arange(F, dtype=np.float32), (F, 1)),
    )
    in_maps = []
    for c in range(N_CORES):
        rows = slice(c * PER, (c + 1) * PER)
        m = dict(common)
        m["xT"] = np.ascontiguousarray(x_sub[rows].T)
        m["invdeg"] = np.ascontiguousarray(inv_deg[rows].reshape(W, 128).T)
        m["dl"] = dls[c]
        for b in range(NQ):
            m[f"gidx{b}"] = gidx[c][b]
        in_maps.append(m)
    return in_maps


def _run(inputs, trace=False):
    x_full = np.asarray(inputs["x_full"], np.float32)
    subset = np.asarray(inputs["subset"], np.int64)
    ei = np.asarray(inputs["edge_index"], np.int64)
    src, dst = ei[0], ei[1]

    x_sub = np.zeros((N_PAD, F), np.float32)
    x_sub[:N_SUB] = x_full[subset]
    deg = np.maximum(np.bincount(dst, minlength=N_SUB).astype(np.float32), 1.0)
    inv_deg = np.ones(N_PAD, np.float32)
    inv_deg[:N_SUB] = 1.0 / deg

    gidx, dls, meta = _prep_edges(src, dst)
    nc = _build_graph(meta)
    in_maps = _make_in_maps(inputs, x_sub, inv_deg, gidx, dls)

    res = run_bass_kernel_spmd(nc, in_maps, list(range(N_CORES)), trace=trace)
    out_rows = np.asarray(res.results[0]["out"]).reshape(LMAX, 2, F)

    fs, ed, alpha = _heads(
        out_rows,
        np.asarray(inputs["W_e1"], np.float32), np.asarray(inputs["b_e1"], np.float32),
        np.asarray(inputs["W_e2"], np.float32), np.asarray(inputs["b_e2"], np.float32),
        np.asarray(inputs["W_h1"], np.float32), np.asarray(inputs["b_h1"], np.float32),
        np.asarray(inputs["W_h2"], np.float32), np.asarray(inputs["b_h2"], np.float32),
    )
    return (fs, ed, alpha), res


def kernel(**inputs):
    (fs, ed, alpha), _ = _run(inputs, trace=False)
    return fs, ed, alpha


# revision 22
# speedup vs baseline: 1.2434x; 1.0406x over previous
"""AdaptiveSAGE GNN message-passing kernel for 8 Trainium2 NeuronCores.

Distribution strategy (dst-sharded message passing, PE-based segment sum):
  - Subgraph nodes padded to N_PAD = 81920 = 8 * 10240; core c owns rows
    [c*10240, (c+1)*10240).
  - The replicated H table is split into 4 quarter-tables of 20480 rows
    (so dma_gather's int16 indices always fit); a host-side node
    permutation maps core c's local rows [q*2560, (q+1)*2560) to quarter
    q at offset c*2560, which makes each quarter exactly one AllGather
    whose input is ready as soon as a quarter of the windows is computed
    -- three of the four AllGathers hide under compute.
  - Edges are assigned to the core owning their destination, bucketed by
    (src quarter, dst window of 128 rows), padded per bucket to a multiple
    of 128 so all cores run one identical instruction stream.
  - Messages are fetched with dma_gather in 1024-index chunks spread over
    the 4 SWDGE queues (all gpsimd Q7 pairs generate descriptors).
  - Segment-sum by destination runs on the TensorEngine: per 128-message
    tile a one-hot(dst) matrix (built by a batched DVE is_equal against an
    iota row) is the stationary matmul operand; one PSUM tile accumulates
    a whole window across all quarters (exact, no RMW races).
  - Messages / one-hots / H table are bf16 (halves gather and AllGather
    bytes, enables fast weight load); PSUM, weights, the local H, and the
    u,v head rows stay f32.
  - The tiny MLP heads (scores / halting probs on rows u=0, v=1) are
    evaluated on the host from the 5 x 2 x 128 head rows the kernel emits.
"""

import numpy as np

import concourse.bass as bass
import concourse.bacc as bacc
import concourse.tile as tile
import concourse.mybir as mybir
from concourse.bass_utils import run_bass_kernel_spmd

F = 128          # feature dim
N_CORES = 8
N_SUB = 80000
PER = 10240      # rows per core
N_PAD = N_CORES * PER
W = PER // F     # dst windows of 128 rows per core
NQ = 4           # quarter tables
QROWS = PER // NQ            # rows per core per quarter
QTAB = N_CORES * QROWS       # rows per quarter table (<= 32767 for int16)
QW = W // NQ                 # windows per quarter
LMAX = 5
CHUNK = 1024     # messages per gather chunk (SWDGE ring caps num_idxs ~<2K)
PAD_DL = 999.0   # out-of-window dst marker for padding slots


def _wrap16(idx: np.ndarray) -> np.ndarray:
    """SWDGE index layout: logical i -> [i%16, i//16], replicated across the
    8 groups of 16 partitions."""
    n = idx.shape[0]
    assert n % 16 == 0
    w = idx.reshape(n // 16, 16).T.astype(np.int16)
    return np.tile(w, (8, 1))


def _prep_edges(src: np.ndarray, dst: np.ndarray):
    """Bucket edges by (dst core, src quarter, dst window); pad each bucket
    to a common multiple-of-128 so the SPMD graph is uniform across cores.

    Gather indices address the permuted quarter tables: node id g with
    c = g // PER, r = g % PER lives in quarter r // QROWS at row
    c * QROWS + (r % QROWS).
    """
    assert QTAB <= 32768
    nw = W
    core_of = dst // PER
    src_c = src // PER
    src_r = src % PER
    bank_of = src_r // QROWS                    # quarter table
    src_idx = src_c * QROWS + (src_r % QROWS)   # row within quarter table
    dst_local = dst - core_of * PER
    w_of = dst_local // F
    run_of = bank_of * nw + w_of

    counts = np.zeros((N_CORES, NQ * nw), dtype=np.int64)
    per_core = []
    for c in range(N_CORES):
        m = core_of == c
        gl = src_idx[m].astype(np.int16)
        dl = (dst_local[m] % F).astype(np.float32)
        rid = run_of[m]
        order = np.argsort(rid, kind="stable")
        gl, dl, rid = gl[order], dl[order], rid[order]
        bounds = np.searchsorted(rid, np.arange(NQ * nw + 1))
        counts[c] = bounds[1:] - bounds[:-1]
        per_core.append((gl, dl, bounds))

    nt = np.ceil(counts.max(axis=0) / F).astype(np.int64)  # tiles per run
    # window-major run order (w, b): a window's bank runs are consecutive,
    # so one PSUM tile accumulates them all
    runs = [(w, b, int(nt[b * nw + w]))
            for w in range(nw) for b in range(NQ) if nt[b * nw + w] > 0]
    # per-bank padded slot counts (bank-major gather layout, window-sorted
    # within each bank) and per-run tile offsets in (w, b) order
    L = [0] * NQ
    slot0 = {}
    for b in range(NQ):
        for w in range(nw):
            n = int(nt[b * nw + w])
            if n:
                slot0[(b, w)] = L[b]
                L[b] += n * F
    tile0 = {}
    tg = 0
    for (w, b, n) in runs:
        tile0[(b, w)] = tg
        tg += n
    T_total = tg

    gidx, dls = [], []
    for c in range(N_CORES):
        gl, dl, bounds = per_core[c]
        gb = [np.zeros(L[b], np.int16) for b in range(NQ)]
        dla = np.full(T_total * F, PAD_DL, np.float32)
        for (w, b, n) in runs:
            r = b * nw + w
            seg = slice(bounds[r], bounds[r + 1])
            cnt = bounds[r + 1] - bounds[r]
            s0 = slot0[(b, w)]
            gb[b][s0:s0 + cnt] = gl[seg]
            t0 = tile0[(b, w)]
            dla[t0 * F:t0 * F + cnt] = dl[seg]
        gidx.append([_wrap16(x) for x in gb])
        dls.append(np.ascontiguousarray(dla.reshape(T_total, F).T))
    meta = dict(L=L, runs=runs, T_total=T_total, slot0=slot0, tile0=tile0)
    return gidx, dls, meta


def _build_graph(meta):
    """Build the SPMD Bass graph (identical for all 8 cores)."""
    L = meta["L"]
    runs = meta["runs"]
    T_total = meta["T_total"]
    slot0 = meta["slot0"]
    tile0 = meta["tile0"]
    f32 = mybir.dt.float32
    bf16 = mybir.dt.bfloat16
    i16 = mybir.dt.int16
    nc = bacc.Bacc("TRN2", target_bir_lowering=False, debug=False,
                   num_devices=N_CORES, num_swdge_queues=4)

    # ---- kernel I/O -------------------------------------------------------
    xT = nc.dram_tensor("xT", [F, PER], f32, kind="ExternalInput")
    invdeg = nc.dram_tensor("invdeg", [F, W], f32, kind="ExternalInput")
    winT = nc.dram_tensor("winT", [F, F], f32, kind="ExternalInput")
    wlT = nc.dram_tensor("wlT", [F, F], f32, kind="ExternalInput")
    wrT = nc.dram_tensor("wrT", [F, F], f32, kind="ExternalInput")
    bin_ = nc.dram_tensor("bin", [F, 1], f32, kind="ExternalInput")
    bl = nc.dram_tensor("bl", [F, 1], f32, kind="ExternalInput")
    ident = nc.dram_tensor("ident", [F, F], f32, kind="ExternalInput")
    iotar = nc.dram_tensor("iotar", [F, F], f32, kind="ExternalInput")
    dl_d = nc.dram_tensor("dl", [F, T_total], f32, kind="ExternalInput")
    gidx_d = [nc.dram_tensor(f"gidx{b}", [128, L[b] // 16], i16,
                             kind="ExternalInput") for b in range(NQ)]
    out = nc.dram_tensor("out", [LMAX, 2, F], f32, kind="ExternalOutput")

    # ---- internal DRAM ----------------------------------------------------
    h_q = [[nc.dram_tensor(f"h_q{p}_{q}", [QTAB, F], bf16,
                           addr_space="Shared") for q in range(NQ)]
           for p in range(2)]
    ag_in = [nc.dram_tensor(f"ag_in{q}", [QROWS, F], bf16) for q in range(NQ)]

    rg = [list(range(N_CORES))]

    with tile.TileContext(nc) as tc:
        with (
            tc.tile_pool(name="sb", bufs=1) as sb,
            tc.tile_pool(name="msgp", bufs=16) as msgp,
            tc.tile_pool(name="ohp", bufs=6) as ohp,
            tc.tile_pool(name="aggtp", bufs=2) as aggtp,
            tc.tile_pool(name="psw", bufs=4, space="PSUM") as pswp,
            tc.tile_pool(name="psh", bufs=2, space="PSUM") as pshp,
            tc.tile_pool(name="psr", bufs=2, space="PSUM") as psrp,
        ):
            # persistent SBUF
            HT = sb.tile([F, PER], f32, tag="HT")       # H local, feature-major
            AGG = sb.tile([128, W, F], f32, tag="AGG")  # scaled agg rows
            HROWB = sb.tile([128, W, F], bf16, tag="HROWB")  # Hnew row-major
            headf = sb.tile([2, F], f32, tag="headf")   # rows u,v at full prec
            w_in = sb.tile([F, F], f32, tag="w_in")
            w_l = sb.tile([F, F], f32, tag="w_l")
            w_r = sb.tile([F, F], f32, tag="w_r")
            b_in = sb.tile([F, 1], f32, tag="b_in")
            b_l = sb.tile([F, 1], f32, tag="b_l")
            idn = sb.tile([F, F], f32, tag="idn")
            iot = sb.tile([F, F], f32, tag="iot")
            ivd = sb.tile([F, W], f32, tag="ivd")
            dlsb = sb.tile([F, T_total], f32, tag="dlsb")
            gsb = [sb.tile([128, L[b] // 16], i16, tag=f"g{b}", name=f"g{b}")
                   for b in range(NQ)]

            def emit_ag(p, q):
                """DMA quarter q's Hnew rows to the bounce and AllGather it
                into quarter table q of table set p."""
                wq = slice(q * QW, (q + 1) * QW)
                nc.sync.dma_start(
                    ag_in[q][:, :].rearrange("(w p) f -> p w f", p=128),
                    HROWB[:, wq, :])
                nc.gpsimd.collective_compute(
                    "AllGather", mybir.AluOpType.bypass, replica_groups=rg,
                    ins=[ag_in[q].ap().opt()], outs=[h_q[p][q].ap().opt()])

            # ---- stage 0: loads -------------------------------------------
            nc.sync.dma_start(w_in[:], winT[:, :])
            nc.sync.dma_start(w_l[:], wlT[:, :])
            nc.sync.dma_start(w_r[:], wrT[:, :])
            nc.sync.dma_start(b_in[:], bin_[:, :])
            nc.sync.dma_start(b_l[:], bl[:, :])
            nc.sync.dma_start(idn[:], ident[:, :])
            nc.sync.dma_start(iot[:], iotar[:, :])
            nc.sync.dma_start(ivd[:], invdeg[:, :])
            nc.sync.dma_start(dlsb[:], dl_d[:, :])
            for b in range(NQ):
                nc.sync.dma_start(gsb[b][:], gidx_d[b][:, :])

            # xT staged through AGG viewed feature-major [F, PER]
            AGGf = AGG[:].rearrange("p w f -> p (w f)")
            nc.sync.dma_start(AGGf, xT[:, :])

            # H0 = W_in @ xT + b_in (feature-major), then row-major; each
            # quarter's AllGather is issued as soon as its windows are done
            for w in range(W):
                ws = slice(w * F, (w + 1) * F)
                ph = pshp.tile([F, F], f32, tag="psh")
                nc.tensor.matmul(ph[:], lhsT=w_in[:], rhs=AGGf[:, ws],
                                 start=True, stop=True)
                nc.vector.tensor_scalar_add(HT[:, ws], ph[:], b_in[:, 0:1])
                pr = psrp.tile([F, F], f32, tag="psr")
                nc.tensor.transpose(pr[:], HT[:, ws], idn[:])
                nc.vector.tensor_copy(HROWB[:, w, :], pr[:])
            for q in range(NQ):
                emit_ag(0, q)

            # ---- steps ----------------------------------------------------
            win_runs = {}
            for (w, b, n) in runs:
                win_runs.setdefault(w, []).append((b, n))
            ntw_max = max(sum(n for (_, n) in rr) for rr in win_runs.values())

            for k in range(LMAX):
                last = k == LMAX - 1
                msg_tiles = {}

                def ensure_chunk(b, j0, k=k, msg_tiles=msg_tiles):
                    if (b, j0) in msg_tiles:
                        return msg_tiles[(b, j0)]
                    n = min(CHUNK, L[b] - j0)
                    msg = msgp.tile([128, CHUNK // 128, F], bf16, tag="msg",
                                    name=f"msg_{k}_{b}_{j0}")
                    cols = slice(j0 // 16, (j0 + n) // 16)
                    nc.gpsimd.dma_gather(
                        out_ap=msg[:, : n // 128, :],
                        in_ap=h_q[k % 2][b][:, :],
                        idxs_ap=gsb[b][:, cols],
                        num_idxs=n, num_idxs_reg=n, elem_size=F)
                    msg_tiles[(b, j0)] = msg
                    return msg

                # window-major: segment-sum + SAGE update per window, so the
                # update pipeline runs underneath the gather stream
                for w in range(W):
                    ws = slice(w * F, (w + 1) * F)
                    rr = win_runs.get(w, [])
                    if rr:
                        ntw = sum(n for (_, n) in rr)
                        tg0 = tile0[(rr[0][0], w)]
                        oh = ohp.tile([128, ntw_max, F], bf16, tag="oh",
                                      name=f"oh_{k}_{w}")
                        nc.vector.tensor_tensor(
                            out=oh[:, :ntw, :],
                            in0=iot[:].unsqueeze(1).to_broadcast([128, ntw, F]),
                            in1=dlsb[:, tg0:tg0 + ntw].unsqueeze(2)
                                .to_broadcast([128, ntw, F]),
                            op=mybir.AluOpType.is_equal)
                        ps = pswp.tile([128, F], f32, tag="psw")
                        ti = 0
                        for (b, n) in rr:
                            s0 = slot0[(b, w)]
                            for t in range(n):
                                s = s0 + t * F
                                msg = ensure_chunk(b, (s // CHUNK) * CHUNK)
                                nc.tensor.matmul(
                                    ps[:], lhsT=oh[:, ti, :],
                                    rhs=msg[:, (s % CHUNK) // F, :],
                                    start=(ti == 0), stop=(ti == ntw - 1))
                                ti += 1
                        # evacuate with the 1/deg scaling folded in
                        nc.vector.tensor_scalar_mul(AGG[:, w, :], ps[:],
                                                    ivd[:, w:w + 1])
                    else:
                        nc.vector.memset(AGG[:, w, :], 0.0)

                    # Hnew_w = relu(W_l @ aggT + W_r @ HT + b_l)
                    pt = pswp.tile([F, F], f32, tag="psw")
                    nc.tensor.transpose(pt[:], AGG[:, w, :], idn[:])
                    at = aggtp.tile([F, F], f32, tag="aggT")
                    nc.vector.tensor_copy(at[:], pt[:])
                    ph = pshp.tile([F, F], f32, tag="psh")
                    nc.tensor.matmul(ph[:], lhsT=w_l[:], rhs=at[:],
                                     start=True, stop=False)
                    nc.tensor.matmul(ph[:], lhsT=w_r[:], rhs=HT[:, ws],
                                     start=False, stop=True)
                    nc.scalar.activation(HT[:, ws], ph[:],
                                         mybir.ActivationFunctionType.Relu,
                                         bias=b_l[:, 0:1])
                    if not last or w == 0:
                        pr = psrp.tile([F, F], f32, tag="psr")
                        nc.tensor.transpose(pr[:], HT[:, ws], idn[:])
                        if w == 0:
                            nc.vector.tensor_copy(headf[:], pr[0:2, :])
                        if not last:
                            nc.vector.tensor_copy(HROWB[:, w, :], pr[:])
                # head rows (global rows 0,1 live on core 0, window 0)
                nc.sync.dma_start(out[k, :, :], headf[:])
                if not last:
                    for q in range(NQ):
                        emit_ag((k + 1) % 2, q)

    # Align each gather's SWDGE queue with the DMASW sem lane Tile assigned
    # (a sem lane must only ever be updated from one queue).
    import re
    for blk in nc.m.functions[0].blocks:
        for ins in blk.instructions:
            if isinstance(ins, mybir.InstDMAGatherAnt) and ins.sync_info:
                m = re.match(r"DMASW(\d+)", ins.sync_info.on_update[0].ant_name)
                if m:
                    ins.queue_num = int(m.group(1)) % 4

    nc.compile()
    return nc


def _heads(out_rows, W_e1, b_e1, W_e2, b_e2, W_h1, b_h1, W_h2, b_h2):
    """Host-side tiny MLP heads, mirroring the reference math in f32."""
    relu = lambda x: np.maximum(x, 0.0)
    alphas, scores = [], []
    p_not = np.float32(1.0)
    for k in range(LMAX):
        h_u = out_rows[k, 0].astype(np.float32)
        h_v = out_rows[k, 1].astype(np.float32)
        feat = np.concatenate([h_u, h_v, h_u * h_v])
        score = relu(feat @ W_e1.T + b_e1) @ W_e2.T + b_e2
        hin = np.concatenate([h_u, h_v, score])
        z = relu(hin @ W_h1.T + b_h1) @ W_h2.T + b_h2
        p_halt = np.float32(1.0) / (np.float32(1.0) + np.exp(-z[0]))
        alphas.append(p_halt * p_not)
        scores.append(score[0])
        p_not = p_not * (np.float32(1.0) - p_halt)
    alpha = np.stack(alphas).astype(np.float32)
    alpha = alpha / (alpha.sum() + np.float32(1e-8))
    scores_v = np.stack(scores).astype(np.float32)
    final_score = (alpha * scores_v).sum()
    depths = np.arange(1, LMAX + 1, dtype=np.float32)
    expected_depth = (alpha * depths).sum()
    return np.float32(final_score), np.float32(expected_depth), alpha


def _make_in_maps(inputs, x_sub, inv_deg, gidx, dls):
    W_in = np.asarray(inputs["W_in"], np.float32)
    W_l = np.asarray(inputs["W_l"], np.float32)
    W_r = np.asarray(inputs["W_r"], np.float32)
    common = dict(
        winT=np.ascontiguousarray(W_in.T),
        wlT=np.ascontiguousarray(W_l.T),
        wrT=np.ascontiguousarray(W_r.T),
        bin=np.asarray(inputs["b_in"], np.float32).reshape(F, 1),
        bl=np.asarray(inputs["b_l"], np.float32).reshape(F, 1),
        ident=np.eye(F, dtype=np.float32),
        iotar=np.tile(np.arange(F, dtype=np.float32), (F, 1)),
    )
    in_maps = []
    for c in range(N_CORES):
        rows = slice(c * PER, (c + 1) * PER)
        m = dict(common)
        m["xT"] = np.ascontiguousarray(x_sub[rows].T)
        m["invdeg"] = np.ascontiguousarray(inv_deg[rows].reshape(W, 128).T)
        m["dl"] = dls[c]
        for b in range(NQ):
            m[f"gidx{b}"] = gidx[c][b]
        in_maps.append(m)
    return in_maps


def _run(inputs, trace=False):
    x_full = np.asarray(inputs["x_full"], np.float32)
    subset = np.asarray(inputs["subset"], np.int64)
    ei = np.asarray(inputs["edge_index"], np.int64)
    src, dst = ei[0], ei[1]

    x_sub = np.zeros((N_PAD, F), np.float32)
    x_sub[:N_SUB] = x_full[subset]
    deg = np.maximum(np.bincount(dst, minlength=N_SUB).astype(np.float32), 1.0)
    inv_deg = np.ones(N_PAD, np.float32)
    inv_deg[:N_SUB] = 1.0 / deg

    gidx, dls, meta = _prep_edges(src, dst)
    nc = _build_graph(meta)
    in_maps = _make_in_maps(inputs, x_sub, inv_deg, gidx, dls)

    res = run_bass_kernel_spmd(nc, in_maps, list(range(N_CORES)), trace=trace)
    out_rows = np.asarray(res.results[0]["out"]).reshape(LMAX, 2, F)

    fs, ed, alpha = _heads(
        out_rows,
        np.asarray(inputs["W_e1"], np.float32), np.asarray(inputs["b_e1"], np.float32),
        np.asarray(inputs["W_e2"], np.float32), np.asarray(inputs["b_e2"], np.float32),
        np.asarray(inputs["W_h1"], np.float32), np.asarray(inputs["b_h1"], np.float32),
        np.asarray(inputs["W_h2"], np.float32), np.asarray(inputs["b_h2"], np.float32),
    )
    return (fs, ed, alpha), res


def kernel(**inputs):
    (fs, ed, alpha), _ = _run(inputs, trace=False)
    return fs, ed, alpha


# revision 23
# speedup vs baseline: 1.2516x; 1.0066x over previous
"""AdaptiveSAGE GNN message-passing kernel for 8 Trainium2 NeuronCores.

Distribution strategy (dst-sharded message passing, PE-based segment sum):
  - Subgraph nodes padded to N_PAD = 81920 = 8 * 10240; core c owns rows
    [c*10240, (c+1)*10240).
  - The replicated H table is split into 4 quarter-tables of 20480 rows
    (so dma_gather's int16 indices always fit); a host-side node
    permutation maps core c's local rows [q*2560, (q+1)*2560) to quarter
    q at offset c*2560, which makes each quarter exactly one AllGather
    whose input is ready as soon as a quarter of the windows is computed
    -- three of the four AllGathers hide under compute.
  - Edges are assigned to the core owning their destination, bucketed by
    (src quarter, dst window of 128 rows), padded per bucket to a multiple
    of 128 so all cores run one identical instruction stream.
  - Messages are fetched with dma_gather in 1024-index chunks spread over
    the 4 SWDGE queues (all gpsimd Q7 pairs generate descriptors).
  - Segment-sum by destination runs on the TensorEngine: per 128-message
    tile a one-hot(dst) matrix (built by a batched DVE is_equal against an
    iota row) is the stationary matmul operand; one PSUM tile accumulates
    a whole window across all quarters (exact, no RMW races).
  - Messages / one-hots / H table are bf16 (halves gather and AllGather
    bytes, enables fast weight load); PSUM, weights, the local H, and the
    u,v head rows stay f32.
  - The tiny MLP heads (scores / halting probs on rows u=0, v=1) are
    evaluated on the host from the 5 x 2 x 128 head rows the kernel emits.
"""

import numpy as np

import concourse.bass as bass
import concourse.bacc as bacc
import concourse.tile as tile
import concourse.mybir as mybir
from concourse.bass_utils import run_bass_kernel_spmd

F = 128          # feature dim
N_CORES = 8
N_SUB = 80000
PER = 10240      # rows per core
N_PAD = N_CORES * PER
W = PER // F     # dst windows of 128 rows per core
NQ = 4           # quarter tables
QROWS = PER // NQ            # rows per core per quarter
QTAB = N_CORES * QROWS       # rows per quarter table (<= 32767 for int16)
QW = W // NQ                 # windows per quarter
LMAX = 5
CHUNK = 1024     # messages per gather chunk (SWDGE ring caps num_idxs ~<2K)
PAD_DL = 999.0   # out-of-window dst marker for padding slots


def _wrap16(idx: np.ndarray) -> np.ndarray:
    """SWDGE index layout: logical i -> [i%16, i//16], replicated across the
    8 groups of 16 partitions."""
    n = idx.shape[0]
    assert n % 16 == 0
    w = idx.reshape(n // 16, 16).T.astype(np.int16)
    return np.tile(w, (8, 1))


def _prep_edges(src: np.ndarray, dst: np.ndarray):
    """Bucket edges by (dst core, src quarter, dst window); pad each bucket
    to a common multiple-of-128 so the SPMD graph is uniform across cores.

    Gather indices address the permuted quarter tables: node id g with
    c = g // PER, r = g % PER lives in quarter r // QROWS at row
    c * QROWS + (r % QROWS).
    """
    assert QTAB <= 32768
    nw = W
    core_of = dst // PER
    src_c = src // PER
    src_r = src % PER
    bank_of = src_r // QROWS                    # quarter table
    src_idx = src_c * QROWS + (src_r % QROWS)   # row within quarter table
    dst_local = dst - core_of * PER
    w_of = dst_local // F
    run_of = bank_of * nw + w_of

    counts = np.zeros((N_CORES, NQ * nw), dtype=np.int64)
    per_core = []
    for c in range(N_CORES):
        m = core_of == c
        gl = src_idx[m].astype(np.int16)
        dl = (dst_local[m] % F).astype(np.float32)
        rid = run_of[m]
        order = np.argsort(rid, kind="stable")
        gl, dl, rid = gl[order], dl[order], rid[order]
        bounds = np.searchsorted(rid, np.arange(NQ * nw + 1))
        counts[c] = bounds[1:] - bounds[:-1]
        per_core.append((gl, dl, bounds))

    nt = np.ceil(counts.max(axis=0) / F).astype(np.int64)  # tiles per run
    # window-major run order (w, b): a window's bank runs are consecutive,
    # so one PSUM tile accumulates them all
    runs = [(w, b, int(nt[b * nw + w]))
            for w in range(nw) for b in range(NQ) if nt[b * nw + w] > 0]
    # per-bank padded slot counts (bank-major gather layout, window-sorted
    # within each bank) and per-run tile offsets in (w, b) order
    L = [0] * NQ
    slot0 = {}
    for b in range(NQ):
        for w in range(nw):
            n = int(nt[b * nw + w])
            if n:
                slot0[(b, w)] = L[b]
                L[b] += n * F
    tile0 = {}
    tg = 0
    for (w, b, n) in runs:
        tile0[(b, w)] = tg
        tg += n
    T_total = tg

    gidx, dls = [], []
    for c in range(N_CORES):
        gl, dl, bounds = per_core[c]
        gb = [np.zeros(L[b], np.int16) for b in range(NQ)]
        dla = np.full(T_total * F, PAD_DL, np.float32)
        for (w, b, n) in runs:
            r = b * nw + w
            seg = slice(bounds[r], bounds[r + 1])
            cnt = bounds[r + 1] - bounds[r]
            s0 = slot0[(b, w)]
            gb[b][s0:s0 + cnt] = gl[seg]
            t0 = tile0[(b, w)]
            dla[t0 * F:t0 * F + cnt] = dl[seg]
        gidx.append([_wrap16(x) for x in gb])
        dls.append(np.ascontiguousarray(dla.reshape(T_total, F).T))
    meta = dict(L=L, runs=runs, T_total=T_total, slot0=slot0, tile0=tile0)
    return gidx, dls, meta


def _build_graph(meta):
    """Build the SPMD Bass graph (identical for all 8 cores)."""
    L = meta["L"]
    runs = meta["runs"]
    T_total = meta["T_total"]
    slot0 = meta["slot0"]
    tile0 = meta["tile0"]
    f32 = mybir.dt.float32
    bf16 = mybir.dt.bfloat16
    i16 = mybir.dt.int16
    nc = bacc.Bacc("TRN2", target_bir_lowering=False, debug=False,
                   num_devices=N_CORES, num_swdge_queues=4)

    # ---- kernel I/O -------------------------------------------------------
    xT = nc.dram_tensor("xT", [F, PER], f32, kind="ExternalInput")
    invdeg = nc.dram_tensor("invdeg", [F, W], f32, kind="ExternalInput")
    winT = nc.dram_tensor("winT", [F, F], f32, kind="ExternalInput")
    wlT = nc.dram_tensor("wlT", [F, F], f32, kind="ExternalInput")
    wrT = nc.dram_tensor("wrT", [F, F], f32, kind="ExternalInput")
    bin_ = nc.dram_tensor("bin", [F, 1], f32, kind="ExternalInput")
    bl = nc.dram_tensor("bl", [F, 1], f32, kind="ExternalInput")
    ident = nc.dram_tensor("ident", [F, F], f32, kind="ExternalInput")
    iotar = nc.dram_tensor("iotar", [F, F], f32, kind="ExternalInput")
    dl_d = nc.dram_tensor("dl", [F, T_total], f32, kind="ExternalInput")
    gidx_d = [nc.dram_tensor(f"gidx{b}", [128, L[b] // 16], i16,
                             kind="ExternalInput") for b in range(NQ)]
    out = nc.dram_tensor("out", [LMAX, 2, F], f32, kind="ExternalOutput")

    # ---- internal DRAM ----------------------------------------------------
    h_q = [[nc.dram_tensor(f"h_q{p}_{q}", [QTAB, F], bf16,
                           addr_space="Shared") for q in range(NQ)]
           for p in range(2)]
    ag_in = [nc.dram_tensor(f"ag_in{q}", [QROWS, F], bf16) for q in range(NQ)]

    rg = [list(range(N_CORES))]

    with tile.TileContext(nc) as tc:
        with (
            tc.tile_pool(name="sb", bufs=1) as sb,
            tc.tile_pool(name="msgp", bufs=20) as msgp,
            tc.tile_pool(name="ohp", bufs=6) as ohp,
            tc.tile_pool(name="aggtp", bufs=2) as aggtp,
            tc.tile_pool(name="psw", bufs=4, space="PSUM") as pswp,
            tc.tile_pool(name="psh", bufs=2, space="PSUM") as pshp,
            tc.tile_pool(name="psr", bufs=2, space="PSUM") as psrp,
        ):
            # persistent SBUF
            HT = sb.tile([F, PER], f32, tag="HT")       # H local, feature-major
            AGG = sb.tile([128, W, F], f32, tag="AGG")  # scaled agg rows
            HROWB = sb.tile([128, W, F], bf16, tag="HROWB")  # Hnew row-major
            headf = sb.tile([2, F], f32, tag="headf")   # rows u,v at full prec
            w_in = sb.tile([F, F], f32, tag="w_in")
            w_l = sb.tile([F, F], f32, tag="w_l")
            w_r = sb.tile([F, F], f32, tag="w_r")
            b_in = sb.tile([F, 1], f32, tag="b_in")
            b_l = sb.tile([F, 1], f32, tag="b_l")
            idn = sb.tile([F, F], f32, tag="idn")
            iot = sb.tile([F, F], f32, tag="iot")
            ivd = sb.tile([F, W], f32, tag="ivd")
            dlsb = sb.tile([F, T_total], f32, tag="dlsb")
            gsb = [sb.tile([128, L[b] // 16], i16, tag=f"g{b}", name=f"g{b}")
                   for b in range(NQ)]

            def emit_ag(p, q):
                """DMA quarter q's Hnew rows to the bounce and AllGather it
                into quarter table q of table set p."""
                wq = slice(q * QW, (q + 1) * QW)
                nc.sync.dma_start(
                    ag_in[q][:, :].rearrange("(w p) f -> p w f", p=128),
                    HROWB[:, wq, :])
                nc.gpsimd.collective_compute(
                    "AllGather", mybir.AluOpType.bypass, replica_groups=rg,
                    ins=[ag_in[q].ap().opt()], outs=[h_q[p][q].ap().opt()])

            # ---- stage 0: loads -------------------------------------------
            nc.sync.dma_start(w_in[:], winT[:, :])
            nc.sync.dma_start(w_l[:], wlT[:, :])
            nc.sync.dma_start(w_r[:], wrT[:, :])
            nc.sync.dma_start(b_in[:], bin_[:, :])
            nc.sync.dma_start(b_l[:], bl[:, :])
            nc.sync.dma_start(idn[:], ident[:, :])
            nc.sync.dma_start(iot[:], iotar[:, :])
            nc.sync.dma_start(ivd[:], invdeg[:, :])
            nc.sync.dma_start(dlsb[:], dl_d[:, :])
            for b in range(NQ):
                nc.sync.dma_start(gsb[b][:], gidx_d[b][:, :])

            # xT staged through AGG viewed feature-major [F, PER]
            AGGf = AGG[:].rearrange("p w f -> p (w f)")
            nc.sync.dma_start(AGGf, xT[:, :])

            # H0 = W_in @ xT + b_in (feature-major), then row-major; each
            # quarter's AllGather is issued as soon as its windows are done
            for w in range(W):
                ws = slice(w * F, (w + 1) * F)
                ph = pshp.tile([F, F], f32, tag="psh")
                nc.tensor.matmul(ph[:], lhsT=w_in[:], rhs=AGGf[:, ws],
                                 start=True, stop=True)
                nc.vector.tensor_scalar_add(HT[:, ws], ph[:], b_in[:, 0:1])
                pr = psrp.tile([F, F], f32, tag="psr")
                nc.tensor.transpose(pr[:], HT[:, ws], idn[:])
                nc.vector.tensor_copy(HROWB[:, w, :], pr[:])
            for q in range(NQ):
                emit_ag(0, q)

            # ---- steps ----------------------------------------------------
            win_runs = {}
            for (w, b, n) in runs:
                win_runs.setdefault(w, []).append((b, n))
            ntw_max = max(sum(n for (_, n) in rr) for rr in win_runs.values())

            for k in range(LMAX):
                last = k == LMAX - 1
                msg_tiles = {}

                def ensure_chunk(b, j0, k=k, msg_tiles=msg_tiles):
                    if (b, j0) in msg_tiles:
                        return msg_tiles[(b, j0)]
                    n = min(CHUNK, L[b] - j0)
                    msg = msgp.tile([128, CHUNK // 128, F], bf16, tag="msg",
                                    name=f"msg_{k}_{b}_{j0}")
                    cols = slice(j0 // 16, (j0 + n) // 16)
                    nc.gpsimd.dma_gather(
                        out_ap=msg[:, : n // 128, :],
                        in_ap=h_q[k % 2][b][:, :],
                        idxs_ap=gsb[b][:, cols],
                        num_idxs=n, num_idxs_reg=n, elem_size=F)
                    msg_tiles[(b, j0)] = msg
                    return msg

                # window-major: segment-sum + SAGE update per window, so the
                # update pipeline runs underneath the gather stream
                for w in range(W):
                    ws = slice(w * F, (w + 1) * F)
                    rr = win_runs.get(w, [])
                    if rr:
                        ntw = sum(n for (_, n) in rr)
                        tg0 = tile0[(rr[0][0], w)]
                        oh = ohp.tile([128, ntw_max, F], bf16, tag="oh",
                                      name=f"oh_{k}_{w}")
                        nc.vector.tensor_tensor(
                            out=oh[:, :ntw, :],
                            in0=iot[:].unsqueeze(1).to_broadcast([128, ntw, F]),
                            in1=dlsb[:, tg0:tg0 + ntw].unsqueeze(2)
                                .to_broadcast([128, ntw, F]),
                            op=mybir.AluOpType.is_equal)
                        ps = pswp.tile([128, F], f32, tag="psw")
                        ti = 0
                        for (b, n) in rr:
                            s0 = slot0[(b, w)]
                            for t in range(n):
                                s = s0 + t * F
                                msg = ensure_chunk(b, (s // CHUNK) * CHUNK)
                                nc.tensor.matmul(
                                    ps[:], lhsT=oh[:, ti, :],
                                    rhs=msg[:, (s % CHUNK) // F, :],
                                    start=(ti == 0), stop=(ti == ntw - 1))
                                ti += 1
                        # evacuate with the 1/deg scaling folded in
                        nc.vector.tensor_scalar_mul(AGG[:, w, :], ps[:],
                                                    ivd[:, w:w + 1])
                    else:
                        nc.vector.memset(AGG[:, w, :], 0.0)

                    # Hnew_w = relu(W_l @ aggT + W_r @ HT + b_l)
                    pt = pswp.tile([F, F], f32, tag="psw")
                    nc.tensor.transpose(pt[:], AGG[:, w, :], idn[:])
                    at = aggtp.tile([F, F], f32, tag="aggT")
                    nc.vector.tensor_copy(at[:], pt[:])
                    ph = pshp.tile([F, F], f32, tag="psh")
                    nc.tensor.matmul(ph[:], lhsT=w_l[:], rhs=at[:],
                                     start=True, stop=False)
                    nc.tensor.matmul(ph[:], lhsT=w_r[:], rhs=HT[:, ws],
                                     start=False, stop=True)
                    nc.scalar.activation(HT[:, ws], ph[:],
                                         mybir.ActivationFunctionType.Relu,
                                         bias=b_l[:, 0:1])
                    if not last or w == 0:
                        pr = psrp.tile([F, F], f32, tag="psr")
                        nc.tensor.transpose(pr[:], HT[:, ws], idn[:])
                        if w == 0:
                            nc.vector.tensor_copy(headf[:], pr[0:2, :])
                        if not last:
                            nc.vector.tensor_copy(HROWB[:, w, :], pr[:])
                # head rows (global rows 0,1 live on core 0, window 0)
                nc.sync.dma_start(out[k, :, :], headf[:])
                if not last:
                    for q in range(NQ):
                        emit_ag((k + 1) % 2, q)

    # Align each gather's SWDGE queue with the DMASW sem lane Tile assigned
    # (a sem lane must only ever be updated from one queue).
    import re
    for blk in nc.m.functions[0].blocks:
        for ins in blk.instructions:
            if isinstance(ins, mybir.InstDMAGatherAnt) and ins.sync_info:
                m = re.match(r"DMASW(\d+)", ins.sync_info.on_update[0].ant_name)
                if m:
                    ins.queue_num = int(m.group(1)) % 4

    nc.compile()
    return nc


def _heads(out_rows, W_e1, b_e1, W_e2, b_e2, W_h1, b_h1, W_h2, b_h2):
    """Host-side tiny MLP heads, mirroring the reference math in f32."""
    relu = lambda x: np.maximum(x, 0.0)
    alphas, scores = [], []
    p_not = np.float32(1.0)
    for k in range(LMAX):
        h_u = out_rows[k, 0].astype(np.float32)
        h_v = out_rows[k, 1].astype(np.float32)
        feat = np.concatenate([h_u, h_v, h_u * h_v])
        score = relu(feat @ W_e1.T + b_e1) @ W_e2.T + b_e2
        hin = np.concatenate([h_u, h_v, score])
        z = relu(hin @ W_h1.T + b_h1) @ W_h2.T + b_h2
        p_halt = np.float32(1.0) / (np.float32(1.0) + np.exp(-z[0]))
        alphas.append(p_halt * p_not)
        scores.append(score[0])
        p_not = p_not * (np.float32(1.0) - p_halt)
    alpha = np.stack(alphas).astype(np.float32)
    alpha = alpha / (alpha.sum() + np.float32(1e-8))
    scores_v = np.stack(scores).astype(np.float32)
    final_score = (alpha * scores_v).sum()
    depths = np.arange(1, LMAX + 1, dtype=np.float32)
    expected_depth = (alpha * depths).sum()
    return np.float32(final_score), np.float32(expected_depth), alpha


def _make_in_maps(inputs, x_sub, inv_deg, gidx, dls):
    W_in = np.asarray(inputs["W_in"], np.float32)
    W_l = np.asarray(inputs["W_l"], np.float32)
    W_r = np.asarray(inputs["W_r"], np.float32)
    common = dict(
        winT=np.ascontiguousarray(W_in.T),
        wlT=np.ascontiguousarray(W_l.T),
        wrT=np.ascontiguousarray(W_r.T),
        bin=np.asarray(inputs["b_in"], np.float32).reshape(F, 1),
        bl=np.asarray(inputs["b_l"], np.float32).reshape(F, 1),
        ident=np.eye(F, dtype=np.float32),
        iotar=np.tile(np.arange(F, dtype=np.float32), (F, 1)),
    )
    in_maps = []
    for c in range(N_CORES):
        rows = slice(c * PER, (c + 1) * PER)
        m = dict(common)
        m["xT"] = np.ascontiguousarray(x_sub[rows].T)
        m["invdeg"] = np.ascontiguousarray(inv_deg[rows].reshape(W, 128).T)
        m["dl"] = dls[c]
        for b in range(NQ):
            m[f"gidx{b}"] = gidx[c][b]
        in_maps.append(m)
    return in_maps


def _run(inputs, trace=False):
    x_full = np.asarray(inputs["x_full"], np.float32)
    subset = np.asarray(inputs["subset"], np.int64)
    ei = np.asarray(inputs["edge_index"], np.int64)
    src, dst = ei[0], ei[1]

    x_sub = np.zeros((N_PAD, F), np.float32)
    x_sub[:N_SUB] = x_full[subset]
    deg = np.maximum(np.bincount(dst, minlength=N_SUB).astype(np.float32), 1.0)
    inv_deg = np.ones(N_PAD, np.float32)
    inv_deg[:N_SUB] = 1.0 / deg

    gidx, dls, meta = _prep_edges(src, dst)
    nc = _build_graph(meta)
    in_maps = _make_in_maps(inputs, x_sub, inv_deg, gidx, dls)

    res = run_bass_kernel_spmd(nc, in_maps, list(range(N_CORES)), trace=trace)
    out_rows = np.asarray(res.results[0]["out"]).reshape(LMAX, 2, F)

    fs, ed, alpha = _heads(
        out_rows,
        np.asarray(inputs["W_e1"], np.float32), np.asarray(inputs["b_e1"], np.float32),
        np.asarray(inputs["W_e2"], np.float32), np.asarray(inputs["b_e2"], np.float32),
        np.asarray(inputs["W_h1"], np.float32), np.asarray(inputs["b_h1"], np.float32),
        np.asarray(inputs["W_h2"], np.float32), np.asarray(inputs["b_h2"], np.float32),
    )
    return (fs, ed, alpha), res


def kernel(**inputs):
    (fs, ed, alpha), _ = _run(inputs, trace=False)
    return fs, ed, alpha


# revision 24
# speedup vs baseline: 1.3239x; 1.0578x over previous
"""AdaptiveSAGE GNN message-passing kernel for 8 Trainium2 NeuronCores.

Distribution strategy (dst-sharded message passing, PE-based segment sum):
  - Subgraph nodes padded to N_PAD = 81920 = 8 * 10240; core c owns rows
    [c*10240, (c+1)*10240).
  - The replicated H table is split into 4 quarter-tables of 20480 rows
    (so dma_gather's int16 indices always fit); a host-side node
    permutation maps core c's local rows [q*2560, (q+1)*2560) to quarter
    q at offset c*2560, so each quarter is exactly one small AllGather.
    Tables are double-buffered across steps: step k gathers from set k%2
    while its four end-of-step AllGathers write set (k+1)%2, letting the
    next step's gathers start as soon as their quarter has arrived.
  - Edges are assigned to the core owning their destination, bucketed by
    (src quarter, dst window of 128 rows), padded per bucket to a multiple
    of 128 so all cores run one identical instruction stream.
  - Messages are fetched with dma_gather in 1024-index chunks spread over
    the 4 SWDGE queues (all gpsimd Q7 pairs generate descriptors).
  - Segment-sum by destination runs on the TensorEngine: per 128-message
    tile a one-hot(dst) matrix (built by a batched DVE is_equal against an
    iota row) is the stationary matmul operand; one PSUM tile accumulates
    a whole window across all quarters (exact, no RMW races).
  - Messages / one-hots / H table are bf16 (halves gather and AllGather
    bytes, enables fast weight load); PSUM, weights, the local H, and the
    u,v head rows stay f32.
  - The tiny MLP heads (scores / halting probs on rows u=0, v=1) are
    evaluated on the host from the 5 x 2 x 128 head rows the kernel emits.
"""

import numpy as np

import concourse.bass as bass
import concourse.bacc as bacc
import concourse.tile as tile
import concourse.mybir as mybir
from concourse.bass_utils import run_bass_kernel_spmd

F = 128          # feature dim
N_CORES = 8
N_SUB = 80000
PER = 10240      # rows per core
N_PAD = N_CORES * PER
W = PER // F     # dst windows of 128 rows per core
NQ = 4           # quarter tables
QROWS = PER // NQ            # rows per core per quarter
QTAB = N_CORES * QROWS       # rows per quarter table (<= 32767 for int16)
QW = W // NQ                 # windows per quarter
LMAX = 5
CHUNK = 1024     # messages per gather chunk (SWDGE ring caps num_idxs ~<2K)
PAD_DL = 999.0   # out-of-window dst marker for padding slots


def _wrap16(idx: np.ndarray) -> np.ndarray:
    """SWDGE index layout: logical i -> [i%16, i//16], replicated across the
    8 groups of 16 partitions."""
    n = idx.shape[0]
    assert n % 16 == 0
    w = idx.reshape(n // 16, 16).T.astype(np.int16)
    return np.tile(w, (8, 1))


def _prep_edges(src: np.ndarray, dst: np.ndarray):
    """Bucket edges by (dst core, src quarter, dst window); pad each bucket
    to a common multiple-of-128 so the SPMD graph is uniform across cores.

    Gather indices address the permuted quarter tables: node id g with
    c = g // PER, r = g % PER lives in quarter r // QROWS at row
    c * QROWS + (r % QROWS).
    """
    assert QTAB <= 32768
    nw = W
    core_of = dst // PER
    src_c = src // PER
    src_r = src % PER
    bank_of = src_r // QROWS                    # quarter table
    src_idx = src_c * QROWS + (src_r % QROWS)   # row within quarter table
    dst_local = dst - core_of * PER
    w_of = dst_local // F
    run_of = bank_of * nw + w_of

    counts = np.zeros((N_CORES, NQ * nw), dtype=np.int64)
    per_core = []
    for c in range(N_CORES):
        m = core_of == c
        gl = src_idx[m].astype(np.int16)
        dl = (dst_local[m] % F).astype(np.float32)
        rid = run_of[m]
        order = np.argsort(rid, kind="stable")
        gl, dl, rid = gl[order], dl[order], rid[order]
        bounds = np.searchsorted(rid, np.arange(NQ * nw + 1))
        counts[c] = bounds[1:] - bounds[:-1]
        per_core.append((gl, dl, bounds))

    nt = np.ceil(counts.max(axis=0) / F).astype(np.int64)  # tiles per run
    # window-major run order (w, b): a window's bank runs are consecutive,
    # so one PSUM tile accumulates them all
    runs = [(w, b, int(nt[b * nw + w]))
            for w in range(nw) for b in range(NQ) if nt[b * nw + w] > 0]
    # per-bank padded slot counts (bank-major gather layout, window-sorted
    # within each bank) and per-run tile offsets in (w, b) order
    L = [0] * NQ
    slot0 = {}
    for b in range(NQ):
        for w in range(nw):
            n = int(nt[b * nw + w])
            if n:
                slot0[(b, w)] = L[b]
                L[b] += n * F
    tile0 = {}
    tg = 0
    for (w, b, n) in runs:
        tile0[(b, w)] = tg
        tg += n
    T_total = tg

    gidx, dls = [], []
    for c in range(N_CORES):
        gl, dl, bounds = per_core[c]
        gb = [np.zeros(L[b], np.int16) for b in range(NQ)]
        dla = np.full(T_total * F, PAD_DL, np.float32)
        for (w, b, n) in runs:
            r = b * nw + w
            seg = slice(bounds[r], bounds[r + 1])
            cnt = bounds[r + 1] - bounds[r]
            s0 = slot0[(b, w)]
            gb[b][s0:s0 + cnt] = gl[seg]
            t0 = tile0[(b, w)]
            dla[t0 * F:t0 * F + cnt] = dl[seg]
        gidx.append([_wrap16(x) for x in gb])
        dls.append(np.ascontiguousarray(dla.reshape(T_total, F).T))
    meta = dict(L=L, runs=runs, T_total=T_total, slot0=slot0, tile0=tile0)
    return gidx, dls, meta


def _build_graph(meta):
    """Build the SPMD Bass graph (identical for all 8 cores)."""
    L = meta["L"]
    runs = meta["runs"]
    T_total = meta["T_total"]
    slot0 = meta["slot0"]
    tile0 = meta["tile0"]
    f32 = mybir.dt.float32
    bf16 = mybir.dt.bfloat16
    i16 = mybir.dt.int16
    nc = bacc.Bacc("TRN2", target_bir_lowering=False, debug=False,
                   num_devices=N_CORES, num_swdge_queues=4)

    # ---- kernel I/O -------------------------------------------------------
    xT = nc.dram_tensor("xT", [F, PER], f32, kind="ExternalInput")
    invdeg = nc.dram_tensor("invdeg", [F, W], f32, kind="ExternalInput")
    winT = nc.dram_tensor("winT", [F, F], f32, kind="ExternalInput")
    wlT = nc.dram_tensor("wlT", [F, F], f32, kind="ExternalInput")
    wrT = nc.dram_tensor("wrT", [F, F], f32, kind="ExternalInput")
    bin_ = nc.dram_tensor("bin", [F, 1], f32, kind="ExternalInput")
    bl = nc.dram_tensor("bl", [F, 1], f32, kind="ExternalInput")
    ident = nc.dram_tensor("ident", [F, F], f32, kind="ExternalInput")
    iotar = nc.dram_tensor("iotar", [F, F], f32, kind="ExternalInput")
    dl_d = nc.dram_tensor("dl", [F, T_total], f32, kind="ExternalInput")
    gidx_d = [nc.dram_tensor(f"gidx{b}", [128, L[b] // 16], i16,
                             kind="ExternalInput") for b in range(NQ)]
    out = nc.dram_tensor("out", [LMAX, 2, F], f32, kind="ExternalOutput")

    # ---- internal DRAM ----------------------------------------------------
    h_q = [[nc.dram_tensor(f"h_q{p}_{q}", [QTAB, F], bf16,
                           addr_space="Shared") for q in range(NQ)]
           for p in range(2)]
    ag_in = [nc.dram_tensor(f"ag_in{q}", [QROWS, F], bf16) for q in range(NQ)]

    rg = [list(range(N_CORES))]

    with tile.TileContext(nc) as tc:
        with (
            tc.tile_pool(name="sb", bufs=1) as sb,
            tc.tile_pool(name="msgp", bufs=20) as msgp,
            tc.tile_pool(name="ohp", bufs=6) as ohp,
            tc.tile_pool(name="aggtp", bufs=2) as aggtp,
            tc.tile_pool(name="psw", bufs=4, space="PSUM") as pswp,
            tc.tile_pool(name="psh", bufs=2, space="PSUM") as pshp,
            tc.tile_pool(name="psr", bufs=2, space="PSUM") as psrp,
        ):
            # persistent SBUF
            HT = sb.tile([F, PER], f32, tag="HT")       # H local, feature-major
            AGG = sb.tile([128, W, F], f32, tag="AGG")  # scaled agg rows
            HROWB = sb.tile([128, W, F], bf16, tag="HROWB")  # Hnew row-major
            headf = sb.tile([2, F], f32, tag="headf")   # rows u,v at full prec
            w_in = sb.tile([F, F], f32, tag="w_in")
            w_l = sb.tile([F, F], f32, tag="w_l")
            w_r = sb.tile([F, F], f32, tag="w_r")
            b_in = sb.tile([F, 1], f32, tag="b_in")
            b_l = sb.tile([F, 1], f32, tag="b_l")
            idn = sb.tile([F, F], f32, tag="idn")
            iot = sb.tile([F, F], f32, tag="iot")
            ivd = sb.tile([F, W], f32, tag="ivd")
            dlsb = sb.tile([F, T_total], f32, tag="dlsb")
            gsb = [sb.tile([128, L[b] // 16], i16, tag=f"g{b}", name=f"g{b}")
                   for b in range(NQ)]

            def emit_ag(p, q):
                """DMA quarter q's Hnew rows to the bounce and AllGather it
                into quarter table q of table set p."""
                wq = slice(q * QW, (q + 1) * QW)
                nc.sync.dma_start(
                    ag_in[q][:, :].rearrange("(w p) f -> p w f", p=128),
                    HROWB[:, wq, :])
                nc.gpsimd.collective_compute(
                    "AllGather", mybir.AluOpType.bypass, replica_groups=rg,
                    ins=[ag_in[q].ap().opt()], outs=[h_q[p][q].ap().opt()])

            # ---- stage 0: loads -------------------------------------------
            nc.sync.dma_start(w_in[:], winT[:, :])
            nc.sync.dma_start(w_l[:], wlT[:, :])
            nc.sync.dma_start(w_r[:], wrT[:, :])
            nc.sync.dma_start(b_in[:], bin_[:, :])
            nc.sync.dma_start(b_l[:], bl[:, :])
            nc.sync.dma_start(idn[:], ident[:, :])
            nc.sync.dma_start(iot[:], iotar[:, :])
            nc.sync.dma_start(ivd[:], invdeg[:, :])
            nc.sync.dma_start(dlsb[:], dl_d[:, :])
            for b in range(NQ):
                nc.sync.dma_start(gsb[b][:], gidx_d[b][:, :])

            # xT staged through AGG viewed feature-major [F, PER]
            AGGf = AGG[:].rearrange("p w f -> p (w f)")
            nc.sync.dma_start(AGGf, xT[:, :])

            # H0 = W_in @ xT + b_in (feature-major), then row-major; each
            # quarter's AllGather is issued as soon as its windows are done
            for w in range(W):
                ws = slice(w * F, (w + 1) * F)
                ph = pshp.tile([F, F], f32, tag="psh")
                nc.tensor.matmul(ph[:], lhsT=w_in[:], rhs=AGGf[:, ws],
                                 start=True, stop=True)
                nc.vector.tensor_scalar_add(HT[:, ws], ph[:], b_in[:, 0:1])
                pr = psrp.tile([F, F], f32, tag="psr")
                nc.tensor.transpose(pr[:], HT[:, ws], idn[:])
                nc.vector.tensor_copy(HROWB[:, w, :], pr[:])
            for q in range(NQ):
                emit_ag(0, q)

            # ---- steps ----------------------------------------------------
            win_runs = {}
            for (w, b, n) in runs:
                win_runs.setdefault(w, []).append((b, n))
            ntw_max = max(sum(n for (_, n) in rr) for rr in win_runs.values())

            for k in range(LMAX):
                last = k == LMAX - 1
                msg_tiles = {}

                def ensure_chunk(b, j0, k=k, msg_tiles=msg_tiles):
                    if (b, j0) in msg_tiles:
                        return msg_tiles[(b, j0)]
                    n = min(CHUNK, L[b] - j0)
                    msg = msgp.tile([128, CHUNK // 128, F], bf16, tag="msg",
                                    name=f"msg_{k}_{b}_{j0}")
                    cols = slice(j0 // 16, (j0 + n) // 16)
                    nc.gpsimd.dma_gather(
                        out_ap=msg[:, : n // 128, :],
                        in_ap=h_q[k % 2][b][:, :],
                        idxs_ap=gsb[b][:, cols],
                        num_idxs=n, num_idxs_reg=n, elem_size=F)
                    msg_tiles[(b, j0)] = msg
                    return msg

                # window-major: segment-sum + SAGE update per window, so the
                # update pipeline runs underneath the gather stream
                for w in range(W):
                    ws = slice(w * F, (w + 1) * F)
                    rr = win_runs.get(w, [])
                    if rr:
                        ntw = sum(n for (_, n) in rr)
                        tg0 = tile0[(rr[0][0], w)]
                        oh = ohp.tile([128, ntw_max, F], bf16, tag="oh",
                                      name=f"oh_{k}_{w}")
                        nc.vector.tensor_tensor(
                            out=oh[:, :ntw, :],
                            in0=iot[:].unsqueeze(1).to_broadcast([128, ntw, F]),
                            in1=dlsb[:, tg0:tg0 + ntw].unsqueeze(2)
                                .to_broadcast([128, ntw, F]),
                            op=mybir.AluOpType.is_equal)
                        ps = pswp.tile([128, F], f32, tag="psw")
                        ti = 0
                        for (b, n) in rr:
                            s0 = slot0[(b, w)]
                            for t in range(n):
                                s = s0 + t * F
                                msg = ensure_chunk(b, (s // CHUNK) * CHUNK)
                                nc.tensor.matmul(
                                    ps[:], lhsT=oh[:, ti, :],
                                    rhs=msg[:, (s % CHUNK) // F, :],
                                    start=(ti == 0), stop=(ti == ntw - 1))
                                ti += 1
                        # evacuate with the 1/deg scaling folded in
                        nc.vector.tensor_scalar_mul(AGG[:, w, :], ps[:],
                                                    ivd[:, w:w + 1])
                    else:
                        nc.vector.memset(AGG[:, w, :], 0.0)

                    # Hnew_w = relu(W_l @ aggT + W_r @ HT + b_l)
                    pt = pswp.tile([F, F], f32, tag="psw")
                    nc.tensor.transpose(pt[:], AGG[:, w, :], idn[:])
                    at = aggtp.tile([F, F], f32, tag="aggT")
                    nc.vector.tensor_copy(at[:], pt[:])
                    ph = pshp.tile([F, F], f32, tag="psh")
                    nc.tensor.matmul(ph[:], lhsT=w_l[:], rhs=at[:],
                                     start=True, stop=False)
                    nc.tensor.matmul(ph[:], lhsT=w_r[:], rhs=HT[:, ws],
                                     start=False, stop=True)
                    nc.scalar.activation(HT[:, ws], ph[:],
                                         mybir.ActivationFunctionType.Relu,
                                         bias=b_l[:, 0:1])
                    if not last or w == 0:
                        pr = psrp.tile([F, F], f32, tag="psr")
                        nc.tensor.transpose(pr[:], HT[:, ws], idn[:])
                        if w == 0:
                            nc.vector.tensor_copy(headf[:], pr[0:2, :])
                        if not last:
                            nc.vector.tensor_copy(HROWB[:, w, :], pr[:])
                # head rows (global rows 0,1 live on core 0, window 0)
                nc.sync.dma_start(out[k, :, :], headf[:])
                if not last:
                    for q in range(NQ):
                        emit_ag((k + 1) % 2, q)

    # Align each gather's SWDGE queue with the DMASW sem lane Tile assigned
    # (a sem lane must only ever be updated from one queue).
    import re
    for blk in nc.m.functions[0].blocks:
        for ins in blk.instructions:
            if isinstance(ins, mybir.InstDMAGatherAnt) and ins.sync_info:
                m = re.match(r"DMASW(\d+)", ins.sync_info.on_update[0].ant_name)
                if m:
                    ins.queue_num = int(m.group(1)) % 4

    nc.compile()
    return nc


def _heads(out_rows, W_e1, b_e1, W_e2, b_e2, W_h1, b_h1, W_h2, b_h2):
    """Host-side tiny MLP heads, mirroring the reference math in f32."""
    relu = lambda x: np.maximum(x, 0.0)
    alphas, scores = [], []
    p_not = np.float32(1.0)
    for k in range(LMAX):
        h_u = out_rows[k, 0].astype(np.float32)
        h_v = out_rows[k, 1].astype(np.float32)
        feat = np.concatenate([h_u, h_v, h_u * h_v])
        score = relu(feat @ W_e1.T + b_e1) @ W_e2.T + b_e2
        hin = np.concatenate([h_u, h_v, score])
        z = relu(hin @ W_h1.T + b_h1) @ W_h2.T + b_h2
        p_halt = np.float32(1.0) / (np.float32(1.0) + np.exp(-z[0]))
        alphas.append(p_halt * p_not)
        scores.append(score[0])
        p_not = p_not * (np.float32(1.0) - p_halt)
    alpha = np.stack(alphas).astype(np.float32)
    alpha = alpha / (alpha.sum() + np.float32(1e-8))
    scores_v = np.stack(scores).astype(np.float32)
    final_score = (alpha * scores_v).sum()
    depths = np.arange(1, LMAX + 1, dtype=np.float32)
    expected_depth = (alpha * depths).sum()
    return np.float32(final_score), np.float32(expected_depth), alpha


def _make_in_maps(inputs, x_sub, inv_deg, gidx, dls):
    W_in = np.asarray(inputs["W_in"], np.float32)
    W_l = np.asarray(inputs["W_l"], np.float32)
    W_r = np.asarray(inputs["W_r"], np.float32)
    common = dict(
        winT=np.ascontiguousarray(W_in.T),
        wlT=np.ascontiguousarray(W_l.T),
        wrT=np.ascontiguousarray(W_r.T),
        bin=np.asarray(inputs["b_in"], np.float32).reshape(F, 1),
        bl=np.asarray(inputs["b_l"], np.float32).reshape(F, 1),
        ident=np.eye(F, dtype=np.float32),
        iotar=np.tile(np.arange(F, dtype=np.float32), (F, 1)),
    )
    in_maps = []
    for c in range(N_CORES):
        rows = slice(c * PER, (c + 1) * PER)
        m = dict(common)
        m["xT"] = np.ascontiguousarray(x_sub[rows].T)
        m["invdeg"] = np.ascontiguousarray(inv_deg[rows].reshape(W, 128).T)
        m["dl"] = dls[c]
        for b in range(NQ):
            m[f"gidx{b}"] = gidx[c][b]
        in_maps.append(m)
    return in_maps


def _run(inputs, trace=False):
    x_full = np.asarray(inputs["x_full"], np.float32)
    subset = np.asarray(inputs["subset"], np.int64)
    ei = np.asarray(inputs["edge_index"], np.int64)
    src, dst = ei[0], ei[1]

    x_sub = np.zeros((N_PAD, F), np.float32)
    x_sub[:N_SUB] = x_full[subset]
    deg = np.maximum(np.bincount(dst, minlength=N_SUB).astype(np.float32), 1.0)
    inv_deg = np.ones(N_PAD, np.float32)
    inv_deg[:N_SUB] = 1.0 / deg

    gidx, dls, meta = _prep_edges(src, dst)
    nc = _build_graph(meta)
    in_maps = _make_in_maps(inputs, x_sub, inv_deg, gidx, dls)

    res = run_bass_kernel_spmd(nc, in_maps, list(range(N_CORES)), trace=trace)
    out_rows = np.asarray(res.results[0]["out"]).reshape(LMAX, 2, F)

    fs, ed, alpha = _heads(
        out_rows,
        np.asarray(inputs["W_e1"], np.float32), np.asarray(inputs["b_e1"], np.float32),
        np.asarray(inputs["W_e2"], np.float32), np.asarray(inputs["b_e2"], np.float32),
        np.asarray(inputs["W_h1"], np.float32), np.asarray(inputs["b_h1"], np.float32),
        np.asarray(inputs["W_h2"], np.float32), np.asarray(inputs["b_h2"], np.float32),
    )
    return (fs, ed, alpha), res


def kernel(**inputs):
    (fs, ed, alpha), _ = _run(inputs, trace=False)
    return fs, ed, alpha


# revision 25
# speedup vs baseline: 1.3510x; 1.0204x over previous
"""AdaptiveSAGE GNN message-passing kernel for 8 Trainium2 NeuronCores.

Distribution strategy (dst-sharded message passing, PE-based segment sum):
  - Subgraph nodes padded to N_PAD = 81920 = 8 * 10240; core c owns rows
    [c*10240, (c+1)*10240).
  - The replicated H table is split into 4 quarter-tables of 20480 rows
    (so dma_gather's int16 indices always fit); a host-side node
    permutation maps core c's local rows [q*2560, (q+1)*2560) to quarter
    q at offset c*2560, so each quarter is exactly one small AllGather.
    Tables are double-buffered across steps: step k gathers from set k%2
    while its four end-of-step AllGathers write set (k+1)%2, letting the
    next step's gathers start as soon as their quarter has arrived.
  - Edges are assigned to the core owning their destination, bucketed by
    (src quarter, dst window of 128 rows), padded per bucket to a multiple
    of 128 so all cores run one identical instruction stream.
  - Messages are fetched with dma_gather in 1024-index chunks spread over
    the 4 SWDGE queues (all gpsimd Q7 pairs generate descriptors).
  - Segment-sum by destination runs on the TensorEngine: per 128-message
    tile a one-hot(dst) matrix (built by a batched DVE is_equal against an
    iota row) is the stationary matmul operand; one PSUM tile accumulates
    a whole window across all quarters (exact, no RMW races).
  - Messages / one-hots / H table are bf16 (halves gather and AllGather
    bytes, enables fast weight load); PSUM, weights, the local H, and the
    u,v head rows stay f32.
  - The tiny MLP heads (scores / halting probs on rows u=0, v=1) are
    evaluated on the host from the 5 x 2 x 128 head rows the kernel emits.
"""

import ml_dtypes
import numpy as np

import concourse.bass as bass
import concourse.bacc as bacc
import concourse.tile as tile
import concourse.mybir as mybir
from concourse.bass_utils import run_bass_kernel_spmd

F = 128          # feature dim
N_CORES = 8
N_SUB = 80000
PER = 10240      # rows per core
N_PAD = N_CORES * PER
W = PER // F     # dst windows of 128 rows per core
NQ = 4           # quarter tables
QROWS = PER // NQ            # rows per core per quarter
QTAB = N_CORES * QROWS       # rows per quarter table (<= 32767 for int16)
QW = W // NQ                 # windows per quarter
LMAX = 5
CHUNK = 1024     # messages per gather chunk (SWDGE ring caps num_idxs ~<2K)
PAD_DL = 999.0   # out-of-window dst marker for padding slots


def _wrap16(idx: np.ndarray) -> np.ndarray:
    """SWDGE index layout: logical i -> [i%16, i//16], replicated across the
    8 groups of 16 partitions."""
    n = idx.shape[0]
    assert n % 16 == 0
    w = idx.reshape(n // 16, 16).T.astype(np.int16)
    return np.tile(w, (8, 1))


def _prep_edges(src: np.ndarray, dst: np.ndarray):
    """Bucket edges by (dst core, src quarter, dst window); pad each bucket
    to a common multiple-of-128 so the SPMD graph is uniform across cores.

    Gather indices address the permuted quarter tables: node id g with
    c = g // PER, r = g % PER lives in quarter r // QROWS at row
    c * QROWS + (r % QROWS).
    """
    assert QTAB <= 32768
    nw = W
    core_of = dst // PER
    src_c = src // PER
    src_r = src % PER
    bank_of = src_r // QROWS                    # quarter table
    src_idx = src_c * QROWS + (src_r % QROWS)   # row within quarter table
    dst_local = dst - core_of * PER
    w_of = dst_local // F
    run_of = bank_of * nw + w_of

    counts = np.zeros((N_CORES, NQ * nw), dtype=np.int64)
    per_core = []
    for c in range(N_CORES):
        m = core_of == c
        gl = src_idx[m].astype(np.int16)
        dl = (dst_local[m] % F).astype(np.float32)
        rid = run_of[m]
        order = np.argsort(rid, kind="stable")
        gl, dl, rid = gl[order], dl[order], rid[order]
        bounds = np.searchsorted(rid, np.arange(NQ * nw + 1))
        counts[c] = bounds[1:] - bounds[:-1]
        per_core.append((gl, dl, bounds))

    nt = np.ceil(counts.max(axis=0) / F).astype(np.int64)  # tiles per run
    # window-major run order (w, b): a window's bank runs are consecutive,
    # so one PSUM tile accumulates them all
    runs = [(w, b, int(nt[b * nw + w]))
            for w in range(nw) for b in range(NQ) if nt[b * nw + w] > 0]
    # per-bank padded slot counts (bank-major gather layout, window-sorted
    # within each bank) and per-run tile offsets in (w, b) order
    L = [0] * NQ
    slot0 = {}
    for b in range(NQ):
        for w in range(nw):
            n = int(nt[b * nw + w])
            if n:
                slot0[(b, w)] = L[b]
                L[b] += n * F
    tile0 = {}
    tg = 0
    for (w, b, n) in runs:
        tile0[(b, w)] = tg
        tg += n
    T_total = tg

    gidx, dls = [], []
    for c in range(N_CORES):
        gl, dl, bounds = per_core[c]
        gb = [np.zeros(L[b], np.int16) for b in range(NQ)]
        dla = np.full(T_total * F, PAD_DL, np.float32)
        for (w, b, n) in runs:
            r = b * nw + w
            seg = slice(bounds[r], bounds[r + 1])
            cnt = bounds[r + 1] - bounds[r]
            s0 = slot0[(b, w)]
            gb[b][s0:s0 + cnt] = gl[seg]
            t0 = tile0[(b, w)]
            dla[t0 * F:t0 * F + cnt] = dl[seg]
        gidx.append([_wrap16(x) for x in gb])
        dls.append(np.ascontiguousarray(dla.reshape(T_total, F).T))
    meta = dict(L=L, runs=runs, T_total=T_total, slot0=slot0, tile0=tile0)
    return gidx, dls, meta


def _build_graph(meta):
    """Build the SPMD Bass graph (identical for all 8 cores)."""
    L = meta["L"]
    runs = meta["runs"]
    T_total = meta["T_total"]
    slot0 = meta["slot0"]
    tile0 = meta["tile0"]
    f32 = mybir.dt.float32
    bf16 = mybir.dt.bfloat16
    i16 = mybir.dt.int16
    nc = bacc.Bacc("TRN2", target_bir_lowering=False, debug=False,
                   num_devices=N_CORES, num_swdge_queues=4)

    # ---- kernel I/O -------------------------------------------------------
    xT = nc.dram_tensor("xT", [F, PER], f32, kind="ExternalInput")
    invdeg = nc.dram_tensor("invdeg", [F, W], f32, kind="ExternalInput")
    winT = nc.dram_tensor("winT", [F, F], f32, kind="ExternalInput")
    wlT = nc.dram_tensor("wlT", [F, F], f32, kind="ExternalInput")
    wrT = nc.dram_tensor("wrT", [F, F], f32, kind="ExternalInput")
    bin_ = nc.dram_tensor("bin", [F, 1], f32, kind="ExternalInput")
    bl = nc.dram_tensor("bl", [F, 1], f32, kind="ExternalInput")
    ident = nc.dram_tensor("ident", [F, F], f32, kind="ExternalInput")
    iotar = nc.dram_tensor("iotar", [F, F], bf16, kind="ExternalInput")
    dl_d = nc.dram_tensor("dl", [F, T_total], bf16, kind="ExternalInput")
    gidx_d = [nc.dram_tensor(f"gidx{b}", [128, L[b] // 16], i16,
                             kind="ExternalInput") for b in range(NQ)]
    out = nc.dram_tensor("out", [LMAX, 2, F], f32, kind="ExternalOutput")

    # ---- internal DRAM ----------------------------------------------------
    h_q = [[nc.dram_tensor(f"h_q{p}_{q}", [QTAB, F], bf16,
                           addr_space="Shared") for q in range(NQ)]
           for p in range(2)]
    ag_in = [nc.dram_tensor(f"ag_in{q}", [QROWS, F], bf16) for q in range(NQ)]

    rg = [list(range(N_CORES))]

    with tile.TileContext(nc) as tc:
        with (
            tc.tile_pool(name="sb", bufs=1) as sb,
            tc.tile_pool(name="msgp", bufs=20) as msgp,
            tc.tile_pool(name="ohp", bufs=6) as ohp,
            tc.tile_pool(name="aggtp", bufs=2) as aggtp,
            tc.tile_pool(name="psw", bufs=4, space="PSUM") as pswp,
            tc.tile_pool(name="psh", bufs=2, space="PSUM") as pshp,
            tc.tile_pool(name="psr", bufs=2, space="PSUM") as psrp,
        ):
            # persistent SBUF
            HT = sb.tile([F, PER], f32, tag="HT")       # H local, feature-major
            AGG = sb.tile([128, W, F], f32, tag="AGG")  # scaled agg rows
            HROWB = sb.tile([128, W, F], bf16, tag="HROWB")  # Hnew row-major
            headf = sb.tile([2, F], f32, tag="headf")   # rows u,v at full prec
            w_in = sb.tile([F, F], f32, tag="w_in")
            w_l = sb.tile([F, F], f32, tag="w_l")
            w_r = sb.tile([F, F], f32, tag="w_r")
            b_in = sb.tile([F, 1], f32, tag="b_in")
            b_l = sb.tile([F, 1], f32, tag="b_l")
            idn = sb.tile([F, F], f32, tag="idn")
            iot = sb.tile([F, F], bf16, tag="iot")
            ivd = sb.tile([F, W], f32, tag="ivd")
            dlsb = sb.tile([F, T_total], bf16, tag="dlsb")
            gsb = [sb.tile([128, L[b] // 16], i16, tag=f"g{b}", name=f"g{b}")
                   for b in range(NQ)]

            def emit_ag(p, q):
                """DMA quarter q's Hnew rows to the bounce and AllGather it
                into quarter table q of table set p."""
                wq = slice(q * QW, (q + 1) * QW)
                nc.sync.dma_start(
                    ag_in[q][:, :].rearrange("(w p) f -> p w f", p=128),
                    HROWB[:, wq, :])
                nc.gpsimd.collective_compute(
                    "AllGather", mybir.AluOpType.bypass, replica_groups=rg,
                    ins=[ag_in[q].ap().opt()], outs=[h_q[p][q].ap().opt()])

            # ---- stage 0: loads -------------------------------------------
            nc.sync.dma_start(w_in[:], winT[:, :])
            nc.sync.dma_start(w_l[:], wlT[:, :])
            nc.sync.dma_start(w_r[:], wrT[:, :])
            nc.sync.dma_start(b_in[:], bin_[:, :])
            nc.sync.dma_start(b_l[:], bl[:, :])
            nc.sync.dma_start(idn[:], ident[:, :])
            nc.sync.dma_start(iot[:], iotar[:, :])
            nc.sync.dma_start(ivd[:], invdeg[:, :])
            nc.sync.dma_start(dlsb[:], dl_d[:, :])
            for b in range(NQ):
                nc.sync.dma_start(gsb[b][:], gidx_d[b][:, :])

            # xT staged through AGG viewed feature-major [F, PER]
            AGGf = AGG[:].rearrange("p w f -> p (w f)")
            nc.sync.dma_start(AGGf, xT[:, :])

            # H0 = W_in @ xT + b_in (feature-major), then row-major; each
            # quarter's AllGather is issued as soon as its windows are done
            for w in range(W):
                ws = slice(w * F, (w + 1) * F)
                ph = pshp.tile([F, F], f32, tag="psh")
                nc.tensor.matmul(ph[:], lhsT=w_in[:], rhs=AGGf[:, ws],
                                 start=True, stop=True)
                nc.vector.tensor_scalar_add(HT[:, ws], ph[:], b_in[:, 0:1])
                pr = psrp.tile([F, F], f32, tag="psr")
                nc.tensor.transpose(pr[:], HT[:, ws], idn[:])
                nc.vector.tensor_copy(HROWB[:, w, :], pr[:])
            for q in range(NQ):
                emit_ag(0, q)

            # ---- steps ----------------------------------------------------
            win_runs = {}
            for (w, b, n) in runs:
                win_runs.setdefault(w, []).append((b, n))
            ntw_max = max(sum(n for (_, n) in rr) for rr in win_runs.values())

            for k in range(LMAX):
                last = k == LMAX - 1
                msg_tiles = {}

                def ensure_chunk(b, j0, k=k, msg_tiles=msg_tiles):
                    if (b, j0) in msg_tiles:
                        return msg_tiles[(b, j0)]
                    n = min(CHUNK, L[b] - j0)
                    msg = msgp.tile([128, CHUNK // 128, F], bf16, tag="msg",
                                    name=f"msg_{k}_{b}_{j0}")
                    cols = slice(j0 // 16, (j0 + n) // 16)
                    nc.gpsimd.dma_gather(
                        out_ap=msg[:, : n // 128, :],
                        in_ap=h_q[k % 2][b][:, :],
                        idxs_ap=gsb[b][:, cols],
                        num_idxs=n, num_idxs_reg=n, elem_size=F)
                    msg_tiles[(b, j0)] = msg
                    return msg

                # window-major: segment-sum + SAGE update per window, so the
                # update pipeline runs underneath the gather stream
                for w in range(W):
                    ws = slice(w * F, (w + 1) * F)
                    rr = win_runs.get(w, [])
                    if rr:
                        ntw = sum(n for (_, n) in rr)
                        tg0 = tile0[(rr[0][0], w)]
                        oh = ohp.tile([128, ntw_max, F], bf16, tag="oh",
                                      name=f"oh_{k}_{w}")
                        nc.vector.tensor_tensor(
                            out=oh[:, :ntw, :],
                            in0=iot[:].unsqueeze(1).to_broadcast([128, ntw, F]),
                            in1=dlsb[:, tg0:tg0 + ntw].unsqueeze(2)
                                .to_broadcast([128, ntw, F]),
                            op=mybir.AluOpType.is_equal)
                        ps = pswp.tile([128, F], f32, tag="psw")
                        ti = 0
                        for (b, n) in rr:
                            s0 = slot0[(b, w)]
                            for t in range(n):
                                s = s0 + t * F
                                msg = ensure_chunk(b, (s // CHUNK) * CHUNK)
                                nc.tensor.matmul(
                                    ps[:], lhsT=oh[:, ti, :],
                                    rhs=msg[:, (s % CHUNK) // F, :],
                                    start=(ti == 0), stop=(ti == ntw - 1))
                                ti += 1
                        # evacuate with the 1/deg scaling folded in
                        nc.vector.tensor_scalar_mul(AGG[:, w, :], ps[:],
                                                    ivd[:, w:w + 1])
                    else:
                        nc.vector.memset(AGG[:, w, :], 0.0)

                    # Hnew_w = relu(W_l @ aggT + W_r @ HT + b_l)
                    pt = pswp.tile([F, F], f32, tag="psw")
                    nc.tensor.transpose(pt[:], AGG[:, w, :], idn[:])
                    at = aggtp.tile([F, F], f32, tag="aggT")
                    nc.vector.tensor_copy(at[:], pt[:])
                    ph = pshp.tile([F, F], f32, tag="psh")
                    nc.tensor.matmul(ph[:], lhsT=w_l[:], rhs=at[:],
                                     start=True, stop=False)
                    nc.tensor.matmul(ph[:], lhsT=w_r[:], rhs=HT[:, ws],
                                     start=False, stop=True)
                    nc.scalar.activation(HT[:, ws], ph[:],
                                         mybir.ActivationFunctionType.Relu,
                                         bias=b_l[:, 0:1])
                    if not last or w == 0:
                        pr = psrp.tile([F, F], f32, tag="psr")
                        nc.tensor.transpose(pr[:], HT[:, ws], idn[:])
                        if w == 0:
                            nc.vector.tensor_copy(headf[:], pr[0:2, :])
                        if not last:
                            nc.vector.tensor_copy(HROWB[:, w, :], pr[:])
                # head rows (global rows 0,1 live on core 0, window 0)
                nc.sync.dma_start(out[k, :, :], headf[:])
                if not last:
                    for q in range(NQ):
                        emit_ag((k + 1) % 2, q)

    # Align each gather's SWDGE queue with the DMASW sem lane Tile assigned
    # (a sem lane must only ever be updated from one queue).
    import re
    for blk in nc.m.functions[0].blocks:
        for ins in blk.instructions:
            if isinstance(ins, mybir.InstDMAGatherAnt) and ins.sync_info:
                m = re.match(r"DMASW(\d+)", ins.sync_info.on_update[0].ant_name)
                if m:
                    ins.queue_num = int(m.group(1)) % 4

    nc.compile()
    return nc


def _heads(out_rows, W_e1, b_e1, W_e2, b_e2, W_h1, b_h1, W_h2, b_h2):
    """Host-side tiny MLP heads, mirroring the reference math in f32."""
    relu = lambda x: np.maximum(x, 0.0)
    alphas, scores = [], []
    p_not = np.float32(1.0)
    for k in range(LMAX):
        h_u = out_rows[k, 0].astype(np.float32)
        h_v = out_rows[k, 1].astype(np.float32)
        feat = np.concatenate([h_u, h_v, h_u * h_v])
        score = relu(feat @ W_e1.T + b_e1) @ W_e2.T + b_e2
        hin = np.concatenate([h_u, h_v, score])
        z = relu(hin @ W_h1.T + b_h1) @ W_h2.T + b_h2
        p_halt = np.float32(1.0) / (np.float32(1.0) + np.exp(-z[0]))
        alphas.append(p_halt * p_not)
        scores.append(score[0])
        p_not = p_not * (np.float32(1.0) - p_halt)
    alpha = np.stack(alphas).astype(np.float32)
    alpha = alpha / (alpha.sum() + np.float32(1e-8))
    scores_v = np.stack(scores).astype(np.float32)
    final_score = (alpha * scores_v).sum()
    depths = np.arange(1, LMAX + 1, dtype=np.float32)
    expected_depth = (alpha * depths).sum()
    return np.float32(final_score), np.float32(expected_depth), alpha


def _make_in_maps(inputs, x_sub, inv_deg, gidx, dls):
    W_in = np.asarray(inputs["W_in"], np.float32)
    W_l = np.asarray(inputs["W_l"], np.float32)
    W_r = np.asarray(inputs["W_r"], np.float32)
    common = dict(
        winT=np.ascontiguousarray(W_in.T),
        wlT=np.ascontiguousarray(W_l.T),
        wrT=np.ascontiguousarray(W_r.T),
        bin=np.asarray(inputs["b_in"], np.float32).reshape(F, 1),
        bl=np.asarray(inputs["b_l"], np.float32).reshape(F, 1),
        ident=np.eye(F, dtype=np.float32),
        iotar=np.tile(np.arange(F, dtype=np.float32),
                      (F, 1)).astype(ml_dtypes.bfloat16),
    )
    in_maps = []
    for c in range(N_CORES):
        rows = slice(c * PER, (c + 1) * PER)
        m = dict(common)
        m["xT"] = np.ascontiguousarray(x_sub[rows].T)
        m["invdeg"] = np.ascontiguousarray(inv_deg[rows].reshape(W, 128).T)
        m["dl"] = dls[c].astype(ml_dtypes.bfloat16)
        for b in range(NQ):
            m[f"gidx{b}"] = gidx[c][b]
        in_maps.append(m)
    return in_maps


def _run(inputs, trace=False):
    x_full = np.asarray(inputs["x_full"], np.float32)
    subset = np.asarray(inputs["subset"], np.int64)
    ei = np.asarray(inputs["edge_index"], np.int64)
    src, dst = ei[0], ei[1]

    x_sub = np.zeros((N_PAD, F), np.float32)
    x_sub[:N_SUB] = x_full[subset]
    deg = np.maximum(np.bincount(dst, minlength=N_SUB).astype(np.float32), 1.0)
    inv_deg = np.ones(N_PAD, np.float32)
    inv_deg[:N_SUB] = 1.0 / deg

    gidx, dls, meta = _prep_edges(src, dst)
    nc = _build_graph(meta)
    in_maps = _make_in_maps(inputs, x_sub, inv_deg, gidx, dls)

    res = run_bass_kernel_spmd(nc, in_maps, list(range(N_CORES)), trace=trace)
    out_rows = np.asarray(res.results[0]["out"]).reshape(LMAX, 2, F)

    fs, ed, alpha = _heads(
        out_rows,
        np.asarray(inputs["W_e1"], np.float32), np.asarray(inputs["b_e1"], np.float32),
        np.asarray(inputs["W_e2"], np.float32), np.asarray(inputs["b_e2"], np.float32),
        np.asarray(inputs["W_h1"], np.float32), np.asarray(inputs["b_h1"], np.float32),
        np.asarray(inputs["W_h2"], np.float32), np.asarray(inputs["b_h2"], np.float32),
    )
    return (fs, ed, alpha), res


def kernel(**inputs):
    (fs, ed, alpha), _ = _run(inputs, trace=False)
    return fs, ed, alpha


# revision 26
# speedup vs baseline: 1.5258x; 1.1294x over previous
"""AdaptiveSAGE GNN message-passing kernel for 8 Trainium2 NeuronCores.

Distribution strategy (dst-sharded message passing, PE-based segment sum):
  - Subgraph nodes padded to N_PAD = 81920 = 8 * 10240; core c owns rows
    [c*10240, (c+1)*10240).
  - The replicated H table is split into 4 quarter-tables of 20480 rows
    (so dma_gather's int16 indices always fit); a host-side node
    permutation maps core c's local rows [q*2560, (q+1)*2560) to quarter
    q at offset c*2560, so each quarter is exactly one small AllGather.
    Tables are double-buffered across steps: step k gathers from set k%2
    while its four end-of-step AllGathers write set (k+1)%2, letting the
    next step's gathers start as soon as their quarter has arrived.
  - Edges are assigned to the core owning their destination, bucketed by
    (src quarter, dst window of 128 rows), padded per bucket to a multiple
    of 128 so all cores run one identical instruction stream.
  - Messages are fetched with dma_gather in 1024-index chunks spread over
    the 4 SWDGE queues (all gpsimd Q7 pairs generate descriptors).
  - Segment-sum by destination runs on the TensorEngine: per 128-message
    tile a one-hot(dst) matrix (built by a batched DVE is_equal against an
    iota row) is the stationary matmul operand; one PSUM tile accumulates
    a whole window across all quarters (exact, no RMW races).
  - Messages / one-hots / H table are bf16 (halves gather and AllGather
    bytes, enables fast weight load); PSUM, weights, the local H, and the
    u,v head rows stay f32.
  - The tiny MLP heads (scores / halting probs on rows u=0, v=1) are
    evaluated on the host from the 5 x 2 x 128 head rows the kernel emits.
"""

import ml_dtypes
import numpy as np

import concourse.bass as bass
import concourse.bacc as bacc
import concourse.tile as tile
import concourse.mybir as mybir
from concourse.bass_utils import run_bass_kernel_spmd

F = 128          # feature dim
N_CORES = 8
N_SUB = 80000
PER = 10240      # rows per core
N_PAD = N_CORES * PER
W = PER // F     # dst windows of 128 rows per core
NQ = 4           # quarter tables
QROWS = PER // NQ            # rows per core per quarter
QTAB = N_CORES * QROWS       # rows per quarter table (<= 32767 for int16)
QW = W // NQ                 # windows per quarter
LMAX = 5
CHUNK = 1024     # messages per gather chunk (SWDGE ring caps num_idxs ~<2K)
PAD_DL = 999.0   # out-of-window dst marker for padding slots


def _wrap16(idx: np.ndarray) -> np.ndarray:
    """SWDGE index layout: logical i -> [i%16, i//16], replicated across the
    8 groups of 16 partitions."""
    n = idx.shape[0]
    assert n % 16 == 0
    w = idx.reshape(n // 16, 16).T.astype(np.int16)
    return np.tile(w, (8, 1))


def _prep_edges(src: np.ndarray, dst: np.ndarray):
    """Bucket edges by (dst core, src quarter, dst window); pad each bucket
    to a common multiple-of-128 so the SPMD graph is uniform across cores.

    Gather indices address the permuted quarter tables: node id g with
    c = g // PER, r = g % PER lives in quarter r // QROWS at row
    c * QROWS + (r % QROWS).
    """
    assert QTAB <= 32768
    nw = W
    core_of = dst // PER
    src_c = src // PER
    src_r = src % PER
    bank_of = src_r // QROWS                    # quarter table
    src_idx = src_c * QROWS + (src_r % QROWS)   # row within quarter table
    dst_local = dst - core_of * PER
    w_of = dst_local // F
    run_of = bank_of * nw + w_of

    counts = np.zeros((N_CORES, NQ * nw), dtype=np.int64)
    per_core = []
    for c in range(N_CORES):
        m = core_of == c
        gl = src_idx[m].astype(np.int16)
        dl = (dst_local[m] % F).astype(np.float32)
        rid = run_of[m]
        order = np.argsort(rid, kind="stable")
        gl, dl, rid = gl[order], dl[order], rid[order]
        bounds = np.searchsorted(rid, np.arange(NQ * nw + 1))
        counts[c] = bounds[1:] - bounds[:-1]
        per_core.append((gl, dl, bounds))

    nt = np.ceil(counts.max(axis=0) / F).astype(np.int64)  # tiles per run
    # window-major run order (w, b): a window's bank runs are consecutive,
    # so one PSUM tile accumulates them all
    runs = [(w, b, int(nt[b * nw + w]))
            for w in range(nw) for b in range(NQ) if nt[b * nw + w] > 0]
    # per-bank padded slot counts (bank-major gather layout, window-sorted
    # within each bank) and per-run tile offsets in (w, b) order
    L = [0] * NQ
    slot0 = {}
    for b in range(NQ):
        for w in range(nw):
            n = int(nt[b * nw + w])
            if n:
                slot0[(b, w)] = L[b]
                L[b] += n * F
    tile0 = {}
    tg = 0
    for (w, b, n) in runs:
        tile0[(b, w)] = tg
        tg += n
    T_total = tg

    gidx, dls = [], []
    for c in range(N_CORES):
        gl, dl, bounds = per_core[c]
        gb = [np.zeros(L[b], np.int16) for b in range(NQ)]
        dla = np.full(T_total * F, PAD_DL, np.float32)
        for (w, b, n) in runs:
            r = b * nw + w
            seg = slice(bounds[r], bounds[r + 1])
            cnt = bounds[r + 1] - bounds[r]
            s0 = slot0[(b, w)]
            gb[b][s0:s0 + cnt] = gl[seg]
            t0 = tile0[(b, w)]
            dla[t0 * F:t0 * F + cnt] = dl[seg]
        gidx.append([_wrap16(x) for x in gb])
        dls.append(np.ascontiguousarray(dla.reshape(T_total, F).T))
    meta = dict(L=L, runs=runs, T_total=T_total, slot0=slot0, tile0=tile0)
    return gidx, dls, meta


def _build_graph(meta):
    """Build the SPMD Bass graph (identical for all 8 cores)."""
    L = meta["L"]
    runs = meta["runs"]
    T_total = meta["T_total"]
    slot0 = meta["slot0"]
    tile0 = meta["tile0"]
    f32 = mybir.dt.float32
    bf16 = mybir.dt.bfloat16
    i16 = mybir.dt.int16
    nc = bacc.Bacc("TRN2", target_bir_lowering=False, debug=False,
                   num_devices=N_CORES, num_swdge_queues=4)

    # ---- kernel I/O -------------------------------------------------------
    xT = nc.dram_tensor("xT", [F, PER], f32, kind="ExternalInput")
    invdeg = nc.dram_tensor("invdeg", [F, W], f32, kind="ExternalInput")
    winT = nc.dram_tensor("winT", [F, F], f32, kind="ExternalInput")
    wlT = nc.dram_tensor("wlT", [F, F], f32, kind="ExternalInput")
    wrT = nc.dram_tensor("wrT", [F, F], f32, kind="ExternalInput")
    bin_ = nc.dram_tensor("bin", [F, 1], f32, kind="ExternalInput")
    bl = nc.dram_tensor("bl", [F, 1], f32, kind="ExternalInput")
    ident = nc.dram_tensor("ident", [F, F], f32, kind="ExternalInput")
    iotar = nc.dram_tensor("iotar", [F, F], bf16, kind="ExternalInput")
    dl_d = nc.dram_tensor("dl", [F, T_total], bf16, kind="ExternalInput")
    gidx_d = [nc.dram_tensor(f"gidx{b}", [128, L[b] // 16], i16,
                             kind="ExternalInput") for b in range(NQ)]
    out = nc.dram_tensor("out", [LMAX, 2, F], f32, kind="ExternalOutput")

    # ---- internal DRAM ----------------------------------------------------
    h_q = [[nc.dram_tensor(f"h_q{p}_{q}", [QTAB, F], bf16,
                           addr_space="Shared") for q in range(NQ)]
           for p in range(2)]
    ag_in = [nc.dram_tensor(f"ag_in{q}", [QROWS, F], bf16) for q in range(NQ)]

    rg = [list(range(N_CORES))]

    with tile.TileContext(nc) as tc:
        with (
            tc.tile_pool(name="sb", bufs=1) as sb,
            tc.tile_pool(name="msgp", bufs=20) as msgp,
            tc.tile_pool(name="ohp", bufs=6) as ohp,
            tc.tile_pool(name="aggtp", bufs=2) as aggtp,
            tc.tile_pool(name="psw", bufs=4, space="PSUM") as pswp,
            tc.tile_pool(name="psh", bufs=2, space="PSUM") as pshp,
            tc.tile_pool(name="psr", bufs=2, space="PSUM") as psrp,
        ):
            # persistent SBUF
            HT = sb.tile([F, PER], f32, tag="HT")       # H local, feature-major
            AGG = sb.tile([128, W, F], f32, tag="AGG")  # scaled agg rows
            HROWB = sb.tile([128, W, F], bf16, tag="HROWB")  # Hnew row-major
            headf = sb.tile([2, F], f32, tag="headf")   # rows u,v at full prec
            w_in = sb.tile([F, F], f32, tag="w_in")
            w_l = sb.tile([F, F], f32, tag="w_l")
            w_r = sb.tile([F, F], f32, tag="w_r")
            b_in = sb.tile([F, 1], f32, tag="b_in")
            b_l = sb.tile([F, 1], f32, tag="b_l")
            idn = sb.tile([F, F], f32, tag="idn")
            iot = sb.tile([F, F], bf16, tag="iot")
            ivd = sb.tile([F, W], f32, tag="ivd")
            dlsb = sb.tile([F, T_total], bf16, tag="dlsb")
            gsb = [sb.tile([128, L[b] // 16], i16, tag=f"g{b}", name=f"g{b}")
                   for b in range(NQ)]

            def emit_ag(p, q):
                """DMA quarter q's Hnew rows to the bounce and AllGather it
                into quarter table q of table set p."""
                wq = slice(q * QW, (q + 1) * QW)
                nc.sync.dma_start(
                    ag_in[q][:, :].rearrange("(w p) f -> p w f", p=128),
                    HROWB[:, wq, :])
                nc.gpsimd.collective_compute(
                    "AllGather", mybir.AluOpType.bypass, replica_groups=rg,
                    ins=[ag_in[q].ap().opt()], outs=[h_q[p][q].ap().opt()])

            # ---- stage 0: loads -------------------------------------------
            nc.sync.dma_start(w_in[:], winT[:, :])
            nc.sync.dma_start(w_l[:], wlT[:, :])
            nc.sync.dma_start(w_r[:], wrT[:, :])
            nc.sync.dma_start(b_in[:], bin_[:, :])
            nc.sync.dma_start(b_l[:], bl[:, :])
            nc.sync.dma_start(idn[:], ident[:, :])
            nc.sync.dma_start(iot[:], iotar[:, :])
            nc.sync.dma_start(ivd[:], invdeg[:, :])
            nc.sync.dma_start(dlsb[:], dl_d[:, :])
            for b in range(NQ):
                nc.sync.dma_start(gsb[b][:], gidx_d[b][:, :])

            # xT staged through AGG viewed feature-major [F, PER]
            AGGf = AGG[:].rearrange("p w f -> p (w f)")
            nc.sync.dma_start(AGGf, xT[:, :])

            # H0 = W_in @ xT + b_in (feature-major), then row-major; each
            # quarter's AllGather is issued as soon as its windows are done
            for w in range(W):
                ws = slice(w * F, (w + 1) * F)
                ph = pshp.tile([F, F], f32, tag="psh")
                nc.tensor.matmul(ph[:], lhsT=w_in[:], rhs=AGGf[:, ws],
                                 start=True, stop=True)
                nc.vector.tensor_scalar_add(HT[:, ws], ph[:], b_in[:, 0:1])
                pr = psrp.tile([F, F], f32, tag="psr")
                nc.tensor.transpose(pr[:], HT[:, ws], idn[:])
                nc.vector.tensor_copy(HROWB[:, w, :], pr[:])
            for q in range(NQ):
                emit_ag(0, q)

            # ---- steps ----------------------------------------------------
            win_runs = {}
            for (w, b, n) in runs:
                win_runs.setdefault(w, []).append((b, n))
            ntw_max = max(sum(n for (_, n) in rr) for rr in win_runs.values())

            for k in range(LMAX):
                last = k == LMAX - 1
                msg_tiles = {}

                def ensure_chunk(b, j0, k=k, msg_tiles=msg_tiles):
                    if (b, j0) in msg_tiles:
                        return msg_tiles[(b, j0)]
                    n = min(CHUNK, L[b] - j0)
                    msg = msgp.tile([128, CHUNK // 128, F], bf16, tag="msg",
                                    name=f"msg_{k}_{b}_{j0}")
                    cols = slice(j0 // 16, (j0 + n) // 16)
                    nc.gpsimd.dma_gather(
                        out_ap=msg[:, : n // 128, :],
                        in_ap=h_q[k % 2][b][:, :],
                        idxs_ap=gsb[b][:, cols],
                        num_idxs=n, num_idxs_reg=n, elem_size=F)
                    msg_tiles[(b, j0)] = msg
                    return msg

                # window-major: segment-sum + SAGE update per window, so the
                # update pipeline runs underneath the gather stream
                for w in range(W):
                    ws = slice(w * F, (w + 1) * F)
                    rr = win_runs.get(w, [])
                    if rr:
                        ntw = sum(n for (_, n) in rr)
                        tg0 = tile0[(rr[0][0], w)]
                        oh = ohp.tile([128, ntw_max, F], bf16, tag="oh",
                                      name=f"oh_{k}_{w}")
                        nc.vector.tensor_tensor(
                            out=oh[:, :ntw, :],
                            in0=iot[:].unsqueeze(1).to_broadcast([128, ntw, F]),
                            in1=dlsb[:, tg0:tg0 + ntw].unsqueeze(2)
                                .to_broadcast([128, ntw, F]),
                            op=mybir.AluOpType.is_equal)
                        ps = pswp.tile([128, F], f32, tag="psw")
                        ti = 0
                        for (b, n) in rr:
                            s0 = slot0[(b, w)]
                            for t in range(n):
                                s = s0 + t * F
                                msg = ensure_chunk(b, (s // CHUNK) * CHUNK)
                                nc.tensor.matmul(
                                    ps[:], lhsT=oh[:, ti, :],
                                    rhs=msg[:, (s % CHUNK) // F, :],
                                    start=(ti == 0), stop=(ti == ntw - 1))
                                ti += 1
                        # evacuate with the 1/deg scaling folded in
                        nc.vector.tensor_scalar_mul(AGG[:, w, :], ps[:],
                                                    ivd[:, w:w + 1])
                    else:
                        nc.vector.memset(AGG[:, w, :], 0.0)

                    # Hnew_w = relu(W_l @ aggT + W_r @ HT + b_l)
                    pt = pswp.tile([F, F], f32, tag="psw")
                    nc.tensor.transpose(pt[:], AGG[:, w, :], idn[:])
                    at = aggtp.tile([F, F], f32, tag="aggT")
                    nc.vector.tensor_copy(at[:], pt[:])
                    ph = pshp.tile([F, F], f32, tag="psh")
                    nc.tensor.matmul(ph[:], lhsT=w_l[:], rhs=at[:],
                                     start=True, stop=False)
                    nc.tensor.matmul(ph[:], lhsT=w_r[:], rhs=HT[:, ws],
                                     start=False, stop=True)
                    nc.scalar.activation(HT[:, ws], ph[:],
                                         mybir.ActivationFunctionType.Relu,
                                         bias=b_l[:, 0:1])
                    if not last or w == 0:
                        pr = psrp.tile([F, F], f32, tag="psr")
                        nc.tensor.transpose(pr[:], HT[:, ws], idn[:])
                        if w == 0:
                            nc.vector.tensor_copy(headf[:], pr[0:2, :])
                        if not last:
                            nc.vector.tensor_copy(HROWB[:, w, :], pr[:])
                # head rows (global rows 0,1 live on core 0, window 0)
                nc.sync.dma_start(out[k, :, :], headf[:])
                if not last:
                    for q in range(NQ):
                        emit_ag((k + 1) % 2, q)

    # Align each gather's SWDGE queue with the DMASW sem lane Tile assigned
    # (a sem lane must only ever be updated from one queue).
    import re
    for blk in nc.m.functions[0].blocks:
        for ins in blk.instructions:
            if isinstance(ins, mybir.InstDMAGatherAnt) and ins.sync_info:
                m = re.match(r"DMASW(\d+)", ins.sync_info.on_update[0].ant_name)
                if m:
                    ins.queue_num = int(m.group(1)) % 4

    nc.compile()
    return nc


def _heads(out_rows, W_e1, b_e1, W_e2, b_e2, W_h1, b_h1, W_h2, b_h2):
    """Host-side tiny MLP heads, mirroring the reference math in f32."""
    relu = lambda x: np.maximum(x, 0.0)
    alphas, scores = [], []
    p_not = np.float32(1.0)
    for k in range(LMAX):
        h_u = out_rows[k, 0].astype(np.float32)
        h_v = out_rows[k, 1].astype(np.float32)
        feat = np.concatenate([h_u, h_v, h_u * h_v])
        score = relu(feat @ W_e1.T + b_e1) @ W_e2.T + b_e2
        hin = np.concatenate([h_u, h_v, score])
        z = relu(hin @ W_h1.T + b_h1) @ W_h2.T + b_h2
        p_halt = np.float32(1.0) / (np.float32(1.0) + np.exp(-z[0]))
        alphas.append(p_halt * p_not)
        scores.append(score[0])
        p_not = p_not * (np.float32(1.0) - p_halt)
    alpha = np.stack(alphas).astype(np.float32)
    alpha = alpha / (alpha.sum() + np.float32(1e-8))
    scores_v = np.stack(scores).astype(np.float32)
    final_score = (alpha * scores_v).sum()
    depths = np.arange(1, LMAX + 1, dtype=np.float32)
    expected_depth = (alpha * depths).sum()
    return np.float32(final_score), np.float32(expected_depth), alpha


def _make_in_maps(inputs, x_sub, inv_deg, gidx, dls):
    W_in = np.asarray(inputs["W_in"], np.float32)
    W_l = np.asarray(inputs["W_l"], np.float32)
    W_r = np.asarray(inputs["W_r"], np.float32)
    common = dict(
        winT=np.ascontiguousarray(W_in.T),
        wlT=np.ascontiguousarray(W_l.T),
        wrT=np.ascontiguousarray(W_r.T),
        bin=np.asarray(inputs["b_in"], np.float32).reshape(F, 1),
        bl=np.asarray(inputs["b_l"], np.float32).reshape(F, 1),
        ident=np.eye(F, dtype=np.float32),
        iotar=np.tile(np.arange(F, dtype=np.float32),
                      (F, 1)).astype(ml_dtypes.bfloat16),
    )
    in_maps = []
    for c in range(N_CORES):
        rows = slice(c * PER, (c + 1) * PER)
        m = dict(common)
        m["xT"] = np.ascontiguousarray(x_sub[rows].T)
        m["invdeg"] = np.ascontiguousarray(inv_deg[rows].reshape(W, 128).T)
        m["dl"] = dls[c].astype(ml_dtypes.bfloat16)
        for b in range(NQ):
            m[f"gidx{b}"] = gidx[c][b]
        in_maps.append(m)
    return in_maps


def _run(inputs, trace=False):
    x_full = np.asarray(inputs["x_full"], np.float32)
    subset = np.asarray(inputs["subset"], np.int64)
    ei = np.asarray(inputs["edge_index"], np.int64)
    src, dst = ei[0], ei[1]

    # Degree-balancing relabel: deal nodes (sorted by in-degree) round-robin
    # into the 640 (core, window) buckets so per-bucket edge counts are even
    # across cores (minimizes the SPMD max-over-cores gather padding).
    # Nodes 0,1 (the u,v heads) stay pinned at rows 0,1 of core 0.
    deg0 = np.bincount(dst, minlength=N_SUB)
    order = np.argsort(-deg0[2:], kind="stable") + 2
    nbuck = N_CORES * W
    perm = np.empty(N_SUB, np.int64)
    perm[0], perm[1] = 0, 1
    bucket = np.arange(len(order)) % nbuck
    rank_in_bucket = np.arange(len(order)) // nbuck
    # bucket 0 already holds nodes 0,1 -> shift its fill positions by 2
    pos = rank_in_bucket + np.where(bucket == 0, 2, 0)
    c_of = bucket // W
    w_of = bucket % W
    perm[order] = c_of * PER + w_of * F + pos
    assert len(np.unique(perm)) == N_SUB and perm.max() < N_PAD

    src = perm[src]
    dst = perm[dst]
    x_sub = np.zeros((N_PAD, F), np.float32)
    x_sub[perm] = x_full[subset]
    deg_p = np.zeros(N_PAD, np.float32)
    deg_p[perm] = np.maximum(deg0, 1).astype(np.float32)
    inv_deg = np.ones(N_PAD, np.float32)
    inv_deg[deg_p > 0] = 1.0 / deg_p[deg_p > 0]

    gidx, dls, meta = _prep_edges(src, dst)
    nc = _build_graph(meta)
    in_maps = _make_in_maps(inputs, x_sub, inv_deg, gidx, dls)

    res = run_bass_kernel_spmd(nc, in_maps, list(range(N_CORES)), trace=trace)
    out_rows = np.asarray(res.results[0]["out"]).reshape(LMAX, 2, F)

    fs, ed, alpha = _heads(
        out_rows,
        np.asarray(inputs["W_e1"], np.float32), np.asarray(inputs["b_e1"], np.float32),
        np.asarray(inputs["W_e2"], np.float32), np.asarray(inputs["b_e2"], np.float32),
        np.asarray(inputs["W_h1"], np.float32), np.asarray(inputs["b_h1"], np.float32),
        np.asarray(inputs["W_h2"], np.float32), np.asarray(inputs["b_h2"], np.float32),
    )
    return (fs, ed, alpha), res


def kernel(**inputs):
    (fs, ed, alpha), _ = _run(inputs, trace=False)
    return fs, ed, alpha
